# revision 1
# baseline (speedup 1.0000x reference)
"""Trainium2 Bass kernel for nn_ConvAttnPool (conv + per-label attention pooling
+ label-graph conv + label-wise scoring), SPMD over 8 NeuronCores.

Sharding: label dim Y=8922 is split 8 ways (1116/core, padded to 1152); the
front-end conv is sharded over batch (1 batch/core) followed by an AllGather of
the conv activations; a second AllGather exchanges the per-label pooled
features (m4t) for the graph conv, whose adjacency is host-pre-transposed,
zero-padded, tiled and bf16-cast so it streams straight into the PE array.
"""
import numpy as np
import ml_dtypes

import concourse.bass as bass
import concourse.bacc as bacc
import concourse.tile as tile
from concourse import mybir
from concourse.bass_utils import run_bass_kernel_spmd

BF16 = ml_dtypes.bfloat16

# problem dims (hardcoded per contract)
B, L, V, E, F, KS, Y = 8, 2500, 50002, 100, 50, 9, 8922
NC = 8
YSV = 1116                   # labels per core (last core has 1110 valid)
YSP = 1152                   # padded labels per core
YT = YSP // 128              # 9 y-tiles
LP = 2560                    # padded seq len
LT = LP // 128               # 20 l-tiles
ZPAD = NC * YSP              # 9216 padded global label dim
ZT = ZPAD // 128             # 72 z-tiles
NBG = B * F                  # 400
VALID = [YSV] * (NC - 1) + [Y - (NC - 1) * YSV]

f32 = mybir.dt.float32
bf16 = mybir.dt.bfloat16
i32 = mybir.dt.int32

_CACHE = {}
import os
PHASES = int(os.environ.get('K_PHASES', '3'))
P3STAGE = int(os.environ.get('K_P3', '3'))


def _build():
    nc = bacc.Bacc("TRN2", target_bir_lowering=False, debug=False,
                   enable_asserts=True, num_devices=NC)

    x_idx = nc.dram_tensor("x_idx", [128, LT], i32, kind="ExternalInput")
    emb_tab = nc.dram_tensor("emb_tab", [V, 128], f32, kind="ExternalInput")
    conv_lhsT = nc.dram_tensor("conv_lhsT", [E, KS * F], bf16, kind="ExternalInput")
    conv_bias = nc.dram_tensor("conv_bias", [F, 1], f32, kind="ExternalInput")
    u4t = nc.dram_tensor("u4t", [F, YSP], bf16, kind="ExternalInput")
    adjt = nc.dram_tensor("adjt", [YT, ZT, 128, 128], bf16, kind="ExternalInput")
    gcn2 = nc.dram_tensor("gcn2", [2 * F, 2 * F], bf16, kind="ExternalInput")
    gcnb_bc = nc.dram_tensor("gcnb_bc", [128, NBG], f32, kind="ExternalInput")
    f4tw = nc.dram_tensor("f4tw", [128, YT * F], bf16, kind="ExternalInput")
    f4w1 = nc.dram_tensor("f4w1", [128, YT * F], bf16, kind="ExternalInput")
    f4w2 = nc.dram_tensor("f4w2", [128, YT * F], bf16, kind="ExternalInput")
    b4t = nc.dram_tensor("b4t", [128, YT], f32, kind="ExternalInput")
    b4 = nc.dram_tensor("b4", [128, YT], f32, kind="ExternalInput")
    ident32 = nc.dram_tensor("ident32", [128, 128], f32, kind="ExternalInput")
    identbf = nc.dram_tensor("identbf", [128, 128], bf16, kind="ExternalInput")
    expmask = nc.dram_tensor("expmask", [128, 1], f32, kind="ExternalInput")
    ones50 = nc.dram_tensor("ones50", [1, F], bf16, kind="ExternalInput")
    outc = nc.dram_tensor("outc", [YSP, 16], f32, kind="ExternalOutput")

    Exp = mybir.ActivationFunctionType.Exp
    Tanh = mybir.ActivationFunctionType.Tanh
    RG = [list(range(NC))]

    with tile.TileContext(nc) as tc:
        with tc.tile_pool(name="const", bufs=1) as cp, \
             tc.tile_pool(name="pers", bufs=1) as pers, \
             tc.tile_pool(name="dram", bufs=1, space="DRAM") as dram:
            ident32_sb = cp.tile([128, 128], f32)
            nc.sync.dma_start(out=ident32_sb[:], in_=ident32[:])
            identbf_sb = cp.tile([128, 128], bf16)
            nc.sync.dma_start(out=identbf_sb[:], in_=identbf[:])
            convw_sb = cp.tile([E, KS * F], bf16)
            nc.sync.dma_start(out=convw_sb[:], in_=conv_lhsT[:])
            convb_sb = cp.tile([F, 1], f32)
            nc.sync.dma_start(out=convb_sb[:], in_=conv_bias[:])
            u4t_sb = cp.tile([F, YSP], bf16)
            nc.sync.dma_start(out=u4t_sb[:], in_=u4t[:])
            gcn2_sb = cp.tile([2 * F, 2 * F], bf16)
            nc.sync.dma_start(out=gcn2_sb[:], in_=gcn2[:])
            gcnb_sb = cp.tile([128, NBG], f32)
            nc.sync.dma_start(out=gcnb_sb[:], in_=gcnb_bc[:])
            f4tw_sb = cp.tile([128, YT * F], bf16)
            nc.sync.dma_start(out=f4tw_sb[:], in_=f4tw[:])
            f4w1_sb = cp.tile([128, YT * F], bf16)
            nc.sync.dma_start(out=f4w1_sb[:], in_=f4w1[:])
            f4w2_sb = cp.tile([128, YT * F], bf16)
            nc.sync.dma_start(out=f4w2_sb[:], in_=f4w2[:])
            b4t_sb = cp.tile([128, YT], f32)
            nc.sync.dma_start(out=b4t_sb[:], in_=b4t[:])
            b4_sb = cp.tile([128, YT], f32)
            nc.sync.dma_start(out=b4_sb[:], in_=b4[:])
            expmask_sb = cp.tile([128, 1], f32)
            nc.sync.dma_start(out=expmask_sb[:], in_=expmask[:])
            ones_sb = cp.tile([1, F], bf16)
            nc.sync.dma_start(out=ones_sb[:], in_=ones50[:])

            m4tT_sb = pers.tile([F, B * YSP], bf16)   # this core's label slice

            HPT_SZ = F * LP               # 128000
            AG1N = HPT_SZ + 128 * LT * 65
            ag1_in = dram.tile([AG1N], bf16)
            ag1_out = dram.tile([NC, AG1N], bf16, addr_space="Shared")

            # ---------------- phase 1: conv on own batch ----------------
            with tc.tile_pool(name="p1", bufs=1) as p1, \
                 tc.tile_pool(name="p1ps", bufs=2, space="PSUM") as p1ps:
                xidx_sb = p1.tile([128, LT], i32)
                nc.sync.dma_start(out=xidx_sb[:], in_=x_idx[:])
                emb_sb = p1.tile([128, LP], f32)
                for lt in range(LT):
                    nc.gpsimd.indirect_dma_start(
                        out=emb_sb[:, lt * 128:(lt + 1) * 128],
                        out_offset=None,
                        in_=emb_tab[:],
                        in_offset=bass.IndirectOffsetOnAxis(
                            ap=xidx_sb[:, lt:lt + 1], axis=0),
                    )
                embT_sb = p1.tile([128, LP + 8], bf16)
                nc.vector.memset(embT_sb[:], 0.0)
                for lt in range(LT):
                    pst = p1ps.tile([128, 128], f32, tag="tp")
                    nc.tensor.transpose(pst[:], emb_sb[:, lt * 128:(lt + 1) * 128],
                                        ident32_sb[:])
                    nc.scalar.copy(out=embT_sb[:, 4 + lt * 128: 4 + (lt + 1) * 128],
                                   in_=pst[:])
                hpT_sb = p1.tile([F, LP], bf16)
                for l5 in range(5):
                    psc = p1ps.tile([F, 512], f32, tag="conv")
                    for k in range(KS):
                        nc.tensor.matmul(
                            psc[:],
                            lhsT=convw_sb[:, k * F:(k + 1) * F],
                            rhs=embT_sb[0:E, l5 * 512 + k: l5 * 512 + k + 512],
                            start=(k == 0), stop=(k == KS - 1))
                    nc.scalar.activation(out=hpT_sb[:, l5 * 512:(l5 + 1) * 512],
                                         in_=psc[:], func=Tanh,
                                         bias=convb_sb[:, 0:1])
                hp1_sb = p1.tile([128, LT * 65], bf16)
                nc.vector.memset(hp1_sb[:], 1.0)
                for lt in range(LT):
                    pst2 = p1ps.tile([128, 64], bf16, tag="tp2")
                    nc.tensor.transpose(pst2[:, 0:F],
                                        hpT_sb[:, lt * 128:(lt + 1) * 128],
                                        identbf_sb[0:F, 0:F])
                    nc.scalar.copy(out=hp1_sb[:, lt * 65:lt * 65 + F],
                                   in_=pst2[:, 0:F])
                nc.sync.dma_start(
                    out=ag1_in[0:HPT_SZ].rearrange("(p n) -> p n", p=F),
                    in_=hpT_sb[:])
                nc.sync.dma_start(
                    out=ag1_in[HPT_SZ:AG1N].rearrange("(p n) -> p n", p=128),
                    in_=hp1_sb[:])
            nc.gpsimd.collective_compute(
                "AllGather", mybir.AluOpType.bypass, replica_groups=RG,
                ins=[ag1_in.opt()], outs=[ag1_out.opt()])

            ag2_in = dram.tile([F * B * YSP], bf16)
            ag2_out = dram.tile([NC, F * B * YSP], bf16, addr_space="Shared")

            # ---------------- phase 2: per-label attention ----------------
            with tc.tile_pool(name="attn", bufs=1) as at, \
                 tc.tile_pool(name="atps", bufs=2, space="PSUM") as atps, \
                 tc.tile_pool(name="atps1", bufs=1, space="PSUM") as atps1:
              if PHASES >= 2:
                hpT_all = at.tile([F, NC * LP], bf16)
                hp1_all = at.tile([128, NC * LT * 65], bf16)
                for r in range(NC):
                    nc.sync.dma_start(
                        out=hpT_all[:, r * LP:(r + 1) * LP],
                        in_=ag1_out[r:r + 1, 0:HPT_SZ].rearrange(
                            "o (p n) -> (o p) n", p=F))
                    nc.sync.dma_start(
                        out=hp1_all[:, r * LT * 65:(r + 1) * LT * 65],
                        in_=ag1_out[r:r + 1, HPT_SZ:AG1N].rearrange(
                            "o (p n) -> (o p) n", p=128))
                for b in range(B):
                    expT_all = at.tile([128, LT * YSP], bf16, tag="expT", bufs=2)
                    for lt in range(LT):
                        psS = atps.tile([128, YSP], f32, tag="S")
                        for c0, cw in ((0, 512), (512, 512), (1024, 128)):
                            nc.tensor.matmul(
                                psS[:, c0:c0 + cw],
                                lhsT=hpT_all[:, b * LP + lt * 128: b * LP + (lt + 1) * 128],
                                rhs=u4t_sb[:, c0:c0 + cw],
                                start=True, stop=True)
                        nc.scalar.activation(
                            out=expT_all[:, lt * YSP:(lt + 1) * YSP],
                            in_=psS[:], func=Exp,
                            bias=(expmask_sb[:, 0:1] if lt == LT - 1 else 0.0))
                    for c0, cw in ((0, 512), (512, 512), (1024, 128)):
                        psM = atps1.tile([65, 512], f32, tag="M")
                        for lt in range(LT):
                            nc.tensor.matmul(
                                psM[:, 0:cw],
                                lhsT=hp1_all[:, (b * LT + lt) * 65:(b * LT + lt + 1) * 65],
                                rhs=expT_all[:, lt * YSP + c0: lt * YSP + c0 + cw],
                                start=(lt == 0), stop=(lt == LT - 1))
                        inv_sb = at.tile([1, 512], bf16, tag="inv", bufs=2)
                        with nc.allow_low_precision(reason="softmax denom bf16"):
                            nc.vector.reciprocal(out=inv_sb[:, 0:cw],
                                                 in_=psM[64:65, 0:cw])
                        psB = atps1.tile([F, 512], f32, tag="Bc")
                        nc.tensor.matmul(psB[:, 0:cw], lhsT=ones_sb[:],
                                         rhs=inv_sb[:, 0:cw], start=True, stop=True)
                        bcast_sb = at.tile([F, 512], bf16, tag="bcast", bufs=2)
                        nc.scalar.copy(out=bcast_sb[:, 0:cw], in_=psB[:, 0:cw])
                        nc.vector.tensor_tensor(
                            out=m4tT_sb[:, b * YSP + c0: b * YSP + c0 + cw],
                            in0=psM[0:F, 0:cw], in1=bcast_sb[:, 0:cw],
                            op=mybir.AluOpType.mult)
            if PHASES >= 2:
              nc.sync.dma_start(
                  out=ag2_in[:].rearrange("(p n) -> p n", p=F), in_=m4tT_sb[:])
              nc.gpsimd.collective_compute(
                  "AllGather", mybir.AluOpType.bypass, replica_groups=RG,
                  ins=[ag2_in.opt()], outs=[ag2_out.opt()])
  
            if PHASES >= 3:
              # ---------------- phase 3: graph conv + label scoring ----------------
              with tc.tile_pool(name="p3", bufs=1) as p3, \
                   tc.tile_pool(name="p3m", bufs=2) as p3m, \
                   tc.tile_pool(name="p3ps", bufs=2, space="PSUM") as p3ps:
                  supp_sb = p3.tile([128, ZT * NBG], bf16)
                  # m4t full, paired batches stacked on partitions for block-diag gcn
                  ag2v = ag2_out.rearrange("r (f b n) -> f r b n", f=F, b=B)
                  for pair in range(B // 2):
                      b0 = 2 * pair
                      mp = p3m.tile([2 * F, ZPAD], bf16, tag="mp")
                      nc.sync.dma_start(
                          out=mp[0:F, :].rearrange("p (r o n) -> p r o n", r=NC, o=1),
                          in_=ag2v[:, :, b0:b0 + 1, :])
                      nc.sync.dma_start(
                          out=mp[F:2 * F, :].rearrange("p (r o n) -> p r o n", r=NC, o=1),
                          in_=ag2v[:, :, b0 + 1:b0 + 2, :])
                      for zt in range(ZT):
                          psU = p3ps.tile([128, 128], f32, tag="U")
                          nc.tensor.matmul(psU[:, 0:2 * F],
                                           lhsT=mp[:, zt * 128:(zt + 1) * 128],
                                           rhs=gcn2_sb[:], start=True, stop=True)
                          nc.vector.tensor_copy(
                              out=supp_sb[:, zt * NBG + b0 * F: zt * NBG + (b0 + 2) * F],
                              in_=psU[:, 0:2 * F])
                  for yt in range(YT if P3STAGE >= 2 else 0):
                      psO = p3ps.tile([128, NBG], f32, tag="O")
                      for zh in range(2):
                          stripe = p3m.tile([128, 36 * 128], bf16, tag="adj")
                          nc.sync.dma_start(
                              out=stripe[:].rearrange("p (t y) -> p t y", t=36),
                              in_=adjt[yt:yt + 1, zh * 36:(zh + 1) * 36].rearrange(
                                  "o t z y -> z (o t) y"))
                          for tl in range(36):
                              zt = zh * 36 + tl
                              nc.tensor.matmul(
                                  psO[:],
                                  lhsT=stripe[:, tl * 128:(tl + 1) * 128],
                                  rhs=supp_sb[:, zt * NBG:(zt + 1) * NBG],
                                  start=(zt == 0), stop=(zt == ZT - 1))
                      o1 = p3.tile([128, NBG], f32, tag="o1", bufs=2)
                      o2 = p3.tile([128, NBG], f32, tag="o2", bufs=2)
                      nc.vector.tensor_tensor(out=o1[:], in0=psO[:], in1=gcnb_sb[:],
                                              op=mybir.AluOpType.add)
                      nc.vector.tensor_scalar_mul(o2[:], o1[:], 0.2)
                      nc.vector.tensor_tensor(out=o1[:], in0=o1[:], in1=o2[:],
                                              op=mybir.AluOpType.max)
                      stage = p3.tile([128, 16], f32, tag="stage", bufs=2)
                      scratch = p3.tile([128, F], f32, tag="scr", bufs=2)
                      tmp1 = p3.tile([128, 1], f32, tag="tmp1", bufs=2)
                      for b in range(B if P3STAGE >= 3 else 0):
                          psT = p3ps.tile([128, 64], bf16, tag="T")
                          nc.tensor.transpose(
                              psT[:, 0:F],
                              m4tT_sb[:, b * YSP + yt * 128: b * YSP + (yt + 1) * 128],
                              identbf_sb[0:F, 0:F])
                          m4t_sb = p3.tile([128, F], bf16, tag="m4t", bufs=2)
                          nc.scalar.copy(out=m4t_sb[:], in_=psT[:, 0:F])
                          tmpa = p3.tile([128, 1], f32, tag="tmpa", bufs=2)
                          nc.vector.tensor_tensor(
                              out=scratch[:], in0=m4t_sb[:],
                              in1=f4tw_sb[:, yt * F:(yt + 1) * F],
                              op=mybir.AluOpType.mult)
                          nc.vector.reduce_sum(out=tmpa[:], in_=scratch[:],
                                               axis=mybir.AxisListType.X)
                          nc.vector.tensor_tensor(
                              out=stage[:, b:b + 1], in0=tmpa[:],
                              in1=b4t_sb[:, yt:yt + 1], op=mybir.AluOpType.add)
                          nc.vector.tensor_tensor(
                              out=scratch[:], in0=m4t_sb[:],
                              in1=f4w1_sb[:, yt * F:(yt + 1) * F],
                              op=mybir.AluOpType.mult)
                          nc.vector.reduce_sum(out=tmp1[:], in_=scratch[:],
                                               axis=mybir.AxisListType.X)
                          nc.vector.tensor_tensor(
                              out=scratch[:], in0=o1[:, b * F:(b + 1) * F],
                              in1=f4w2_sb[:, yt * F:(yt + 1) * F],
                              op=mybir.AluOpType.mult)
                          nc.vector.reduce_sum(out=tmpa[:], in_=scratch[:],
                                               axis=mybir.AxisListType.X)
                          nc.vector.tensor_tensor(
                              out=tmp1[:], in0=tmp1[:], in1=tmpa[:],
                              op=mybir.AluOpType.add)
                          nc.vector.tensor_tensor(
                              out=stage[:, 8 + b:9 + b], in0=tmp1[:],
                              in1=b4_sb[:, yt:yt + 1], op=mybir.AluOpType.add)
                      if P3STAGE < 3:
                          nc.vector.tensor_copy(out=stage[:], in_=o1[:, 0:16])
                      nc.sync.dma_start(out=outc[yt * 128:(yt + 1) * 128, :],
                                        in_=stage[:])

    nc.compile()
    return nc


def _bf(x):
    return np.ascontiguousarray(np.asarray(x, dtype=np.float32).astype(BF16))


def _prep_inputs(x, embed_w, conv_w, conv_b, U4_w, gcn_w, gcn_b, adj,
                 final4t_w, final4t_b, final4_w, final4_b):
    x = np.asarray(x).astype(np.int64)
    embed_w = np.asarray(embed_w, dtype=np.float32)
    conv_w = np.asarray(conv_w, dtype=np.float32)
    conv_b = np.asarray(conv_b, dtype=np.float32)
    U4_w = np.asarray(U4_w, dtype=np.float32)
    gcn_w = np.asarray(gcn_w, dtype=np.float32)
    gcn_b = np.asarray(gcn_b, dtype=np.float32)
    adj = np.asarray(adj, dtype=np.float32)
    f4t_w = np.asarray(final4t_w, dtype=np.float32)
    f4t_b = np.asarray(final4t_b, dtype=np.float32)
    f4_w = np.asarray(final4_w, dtype=np.float32)
    f4_b = np.asarray(final4_b, dtype=np.float32)

    emb_tab = np.zeros((V, 128), np.float32)
    emb_tab[:, :E] = embed_w
    conv_lhsT = np.zeros((E, KS * F), np.float32)
    for k in range(KS):
        conv_lhsT[:, k * F:(k + 1) * F] = conv_w[:, :, k].T
    conv_lhsT = _bf(conv_lhsT)
    conv_bias = np.ascontiguousarray(conv_b.reshape(F, 1))
    gcn2 = np.zeros((2 * F, 2 * F), np.float32)
    gcn2[:F, :F] = gcn_w
    gcn2[F:, F:] = gcn_w
    gcn2 = _bf(gcn2)
    gcnb_bc = np.ascontiguousarray(
        np.broadcast_to(np.tile(gcn_b, B)[None, :], (128, NBG)))
    ident32 = np.eye(128, dtype=np.float32)
    identbf = _bf(np.eye(128, dtype=np.float32))
    expmask = np.zeros((128, 1), np.float32)
    expmask[L - (LT - 1) * 128:, 0] = -30000.0
    ones50 = _bf(np.ones((1, F), np.float32))

    shared = dict(emb_tab=emb_tab, conv_lhsT=conv_lhsT, conv_bias=conv_bias,
                  gcn2=gcn2, gcnb_bc=gcnb_bc, ident32=ident32, identbf=identbf,
                  expmask=expmask, ones50=ones50)

    in_maps = []
    for c in range(NC):
        v = VALID[c]
        x_idx = np.zeros((128, LT), np.int32)
        xp = np.zeros(LP, np.int32)
        xp[:L] = x[c]
        x_idx[:, :] = xp.reshape(LT, 128).T

        u4t_c = np.zeros((F, YSP), np.float32)
        u4t_c[:, :v] = U4_w[c * YSV:c * YSV + v].T

        at = np.zeros((ZPAD, YSP), np.float32)
        for blk in range(NC):
            vb = VALID[blk]
            at[blk * YSP:blk * YSP + vb, :v] = adj[c * YSV:c * YSV + v,
                                                   blk * YSV:blk * YSV + vb].T
        at = at.astype(BF16)
        adjt_c = np.ascontiguousarray(
            at.reshape(ZT, 128, YT, 128).transpose(2, 0, 1, 3))

        def rowpack(w):
            out = np.zeros((128, YT * F), np.float32)
            wp = np.zeros((YSP, F), np.float32)
            wp[:v] = w[c * YSV:c * YSV + v]
            for yt in range(YT):
                out[:, yt * F:(yt + 1) * F] = wp[yt * 128:(yt + 1) * 128]
            return _bf(out)

        def biaspack(bias):
            out = np.zeros((128, YT), np.float32)
            bp = np.zeros(YSP, np.float32)
            bp[:v] = bias[c * YSV:c * YSV + v]
            out[:, :] = bp.reshape(YT, 128).T
            return np.ascontiguousarray(out)

        m = dict(shared)
        m.update(x_idx=x_idx, u4t=_bf(u4t_c), adjt=adjt_c,
                 f4tw=rowpack(f4t_w), f4w1=rowpack(f4_w[:, :F]),
                 f4w2=rowpack(f4_w[:, F:]), b4t=biaspack(f4t_b),
                 b4=biaspack(f4_b))
        in_maps.append(m)
    return in_maps


def _postprocess(results):
    y4t = np.zeros((B, Y), np.float32)
    y4 = np.zeros((B, Y), np.float32)
    for c in range(NC):
        v = VALID[c]
        oc = results[c]["outc"]
        y4t[:, c * YSV:c * YSV + v] = oc[:v, 0:8].T
        y4[:, c * YSV:c * YSV + v] = oc[:v, 8:16].T
    return y4t, y4


def _get_nc():
    if "nc" not in _CACHE:
        _CACHE["nc"] = _build()
    return _CACHE["nc"]


def run_raw(in_maps, **kw):
    nc = _get_nc()
    return run_bass_kernel_spmd(nc, in_maps, list(range(NC)), **kw)


def kernel(x, target, embed_w, conv_w, conv_b, U4_w, gcn_w, gcn_b, adj,
           final4t_w, final4t_b, final4_w, final4_b):
    in_maps = _prep_inputs(x, embed_w, conv_w, conv_b, U4_w, gcn_w, gcn_b, adj,
                           final4t_w, final4t_b, final4_w, final4_b)
    res = run_raw(in_maps)
    return _postprocess(res.results)



# revision 2
# speedup vs baseline: 7485.7558x; 7485.7558x over previous
"""Trainium2 Bass kernel for nn_ConvAttnPool (conv + per-label attention pooling
+ label-graph conv + label-wise scoring), SPMD over 8 NeuronCores.

Sharding: label dim Y=8922 is split 8 ways (1116/core, padded to 1152); the
front-end conv is sharded over batch (1 batch/core) followed by an AllGather of
the conv activations; a second AllGather exchanges the per-label pooled
features (m4t) for the graph conv.

Host-side prep keeps the per-call device payload small (the axon PJRT tunnel
is ~70 MB/s): the embedding lookup is resolved on host (ship the 0.66 MB
gathered+transposed activations instead of the 25.6 MB vocab table per core),
and the adjacency is pre-transposed, tiled, scaled by a power of two s and
stored fp8e4 (TRN E4M3, max 240) — the matching 1/s is folded into the gcn
weight so the device computes exactly adj @ support with no extra ops.
"""
import numpy as np
import ml_dtypes

import concourse.bass as bass
import concourse.bacc as bacc
import concourse.tile as tile
from concourse import mybir
from concourse.bass_utils import run_bass_kernel_spmd

BF16 = ml_dtypes.bfloat16
FP8 = ml_dtypes.float8_e4m3

# problem dims (hardcoded per contract)
B, L, V, E, F, KS, Y = 8, 2500, 50002, 100, 50, 9, 8922
NC = 8
YSV = 1116                   # labels per core (last core has 1110 valid)
YSP = 1152                   # padded labels per core
YT = YSP // 128              # 9 y-tiles
LP = 2560                    # padded seq len
LT = LP // 128               # 20 l-tiles
ZPAD = NC * YSP              # 9216 padded global label dim
ZT = ZPAD // 128             # 72 z-tiles
NBG = B * F                  # 400
VALID = [YSV] * (NC - 1) + [Y - (NC - 1) * YSV]

f32 = mybir.dt.float32
bf16 = mybir.dt.bfloat16
fp8 = mybir.dt.float8e4
i32 = mybir.dt.int32

_CACHE = {}
import os
ADJ_MIXED = int(os.environ.get('K_ADJ_MIXED', '0'))


def _build():
    nc = bacc.Bacc("TRN2", target_bir_lowering=False, debug=False,
                   enable_asserts=True, num_devices=NC)

    embT = nc.dram_tensor("embT", [128, LP + 8], bf16, kind="ExternalInput")
    conv_lhsT = nc.dram_tensor("conv_lhsT", [E, KS * F], bf16, kind="ExternalInput")
    conv_bias = nc.dram_tensor("conv_bias", [F, 1], f32, kind="ExternalInput")
    u4t = nc.dram_tensor("u4t", [F, YSP], bf16, kind="ExternalInput")
    adjt = nc.dram_tensor("adjt", [YT, ZT, 128, 128], fp8, kind="ExternalInput")
    gcn2 = nc.dram_tensor("gcn2", [2 * F, 2 * F], bf16, kind="ExternalInput")
    gcnb_bc = nc.dram_tensor("gcnb_bc", [128, NBG], f32, kind="ExternalInput")
    f4tw = nc.dram_tensor("f4tw", [128, YT * F], bf16, kind="ExternalInput")
    f4w1 = nc.dram_tensor("f4w1", [128, YT * F], bf16, kind="ExternalInput")
    f4w2 = nc.dram_tensor("f4w2", [128, YT * F], bf16, kind="ExternalInput")
    b4t = nc.dram_tensor("b4t", [128, YT], f32, kind="ExternalInput")
    b4 = nc.dram_tensor("b4", [128, YT], f32, kind="ExternalInput")
    identbf = nc.dram_tensor("identbf", [128, 128], bf16, kind="ExternalInput")
    expmask = nc.dram_tensor("expmask", [128, 1], f32, kind="ExternalInput")
    ones50 = nc.dram_tensor("ones50", [1, F], bf16, kind="ExternalInput")
    outc = nc.dram_tensor("outc", [YSP, 16], f32, kind="ExternalOutput")

    Exp = mybir.ActivationFunctionType.Exp
    Tanh = mybir.ActivationFunctionType.Tanh
    RG = [list(range(NC))]

    with tile.TileContext(nc) as tc:
        with tc.tile_pool(name="const", bufs=1) as cp, \
             tc.tile_pool(name="pers", bufs=1) as pers, \
             tc.tile_pool(name="dram", bufs=1, space="DRAM") as dram:
            identbf_sb = cp.tile([128, 128], bf16)
            nc.sync.dma_start(out=identbf_sb[:], in_=identbf[:])
            convw_sb = cp.tile([E, KS * F], bf16)
            nc.sync.dma_start(out=convw_sb[:], in_=conv_lhsT[:])
            convb_sb = cp.tile([F, 1], f32)
            nc.sync.dma_start(out=convb_sb[:], in_=conv_bias[:])
            u4t_sb = cp.tile([F, YSP], bf16)
            nc.sync.dma_start(out=u4t_sb[:], in_=u4t[:])
            gcn2_sb = cp.tile([2 * F, 2 * F], bf16)
            nc.sync.dma_start(out=gcn2_sb[:], in_=gcn2[:])
            gcnb_sb = cp.tile([128, NBG], f32)
            nc.sync.dma_start(out=gcnb_sb[:], in_=gcnb_bc[:])
            f4tw_sb = cp.tile([128, YT * F], bf16)
            nc.sync.dma_start(out=f4tw_sb[:], in_=f4tw[:])
            f4w1_sb = cp.tile([128, YT * F], bf16)
            nc.sync.dma_start(out=f4w1_sb[:], in_=f4w1[:])
            f4w2_sb = cp.tile([128, YT * F], bf16)
            nc.sync.dma_start(out=f4w2_sb[:], in_=f4w2[:])
            b4t_sb = cp.tile([128, YT], f32)
            nc.sync.dma_start(out=b4t_sb[:], in_=b4t[:])
            b4_sb = cp.tile([128, YT], f32)
            nc.sync.dma_start(out=b4_sb[:], in_=b4[:])
            expmask_sb = cp.tile([128, 1], f32)
            nc.sync.dma_start(out=expmask_sb[:], in_=expmask[:])
            ones_sb = cp.tile([1, F], bf16)
            nc.sync.dma_start(out=ones_sb[:], in_=ones50[:])

            m4tT_sb = pers.tile([F, B * YSP], bf16)   # this core's label slice

            HPT_SZ = F * LP               # 128000
            AG1N = HPT_SZ + 128 * LT * 65
            ag1_in = dram.tile([AG1N], bf16)
            ag1_out = dram.tile([NC, AG1N], bf16, addr_space="Shared")

            # ---------------- phase 1: conv on own batch ----------------
            with tc.tile_pool(name="p1", bufs=1) as p1, \
                 tc.tile_pool(name="p1ps", bufs=2, space="PSUM") as p1ps:
                embT_sb = p1.tile([128, LP + 8], bf16)
                nc.sync.dma_start(out=embT_sb[:], in_=embT[:])
                hpT_sb = p1.tile([F, LP], bf16)
                for l5 in range(5):
                    psc = p1ps.tile([F, 512], f32, tag="conv")
                    for k in range(KS):
                        nc.tensor.matmul(
                            psc[:],
                            lhsT=convw_sb[:, k * F:(k + 1) * F],
                            rhs=embT_sb[0:E, l5 * 512 + k: l5 * 512 + k + 512],
                            start=(k == 0), stop=(k == KS - 1))
                    nc.scalar.activation(out=hpT_sb[:, l5 * 512:(l5 + 1) * 512],
                                         in_=psc[:], func=Tanh,
                                         bias=convb_sb[:, 0:1])
                hp1_sb = p1.tile([128, LT * 65], bf16)
                nc.vector.memset(hp1_sb[:], 1.0)
                for lt in range(LT):
                    pst2 = p1ps.tile([128, 64], bf16, tag="tp2")
                    nc.tensor.transpose(pst2[:, 0:F],
                                        hpT_sb[:, lt * 128:(lt + 1) * 128],
                                        identbf_sb[0:F, 0:F])
                    nc.scalar.copy(out=hp1_sb[:, lt * 65:lt * 65 + F],
                                   in_=pst2[:, 0:F])
                nc.sync.dma_start(
                    out=ag1_in[0:HPT_SZ].rearrange("(p n) -> p n", p=F),
                    in_=hpT_sb[:])
                nc.sync.dma_start(
                    out=ag1_in[HPT_SZ:AG1N].rearrange("(p n) -> p n", p=128),
                    in_=hp1_sb[:])
            nc.gpsimd.collective_compute(
                "AllGather", mybir.AluOpType.bypass, replica_groups=RG,
                ins=[ag1_in.opt()], outs=[ag1_out.opt()])

            ag2_in = dram.tile([F * B * YSP], bf16)
            ag2_out = dram.tile([NC, F * B * YSP], bf16, addr_space="Shared")

            # ---------------- phase 2: per-label attention ----------------
            with tc.tile_pool(name="attn", bufs=1) as at, \
                 tc.tile_pool(name="atps", bufs=2, space="PSUM") as atps, \
                 tc.tile_pool(name="atps1", bufs=1, space="PSUM") as atps1:
                hpT_all = at.tile([F, NC * LP], bf16)
                hp1_all = at.tile([128, NC * LT * 65], bf16)
                for r in range(NC):
                    nc.sync.dma_start(
                        out=hpT_all[:, r * LP:(r + 1) * LP],
                        in_=ag1_out[r:r + 1, 0:HPT_SZ].rearrange(
                            "o (p n) -> (o p) n", p=F))
                    nc.sync.dma_start(
                        out=hp1_all[:, r * LT * 65:(r + 1) * LT * 65],
                        in_=ag1_out[r:r + 1, HPT_SZ:AG1N].rearrange(
                            "o (p n) -> (o p) n", p=128))
                for b in range(B):
                    expT_all = at.tile([128, LT * YSP], bf16, tag="expT", bufs=2)
                    for lt in range(LT):
                        psS = atps.tile([128, YSP], f32, tag="S")
                        for c0, cw in ((0, 512), (512, 512), (1024, 128)):
                            nc.tensor.matmul(
                                psS[:, c0:c0 + cw],
                                lhsT=hpT_all[:, b * LP + lt * 128: b * LP + (lt + 1) * 128],
                                rhs=u4t_sb[:, c0:c0 + cw],
                                start=True, stop=True)
                        nc.scalar.activation(
                            out=expT_all[:, lt * YSP:(lt + 1) * YSP],
                            in_=psS[:], func=Exp,
                            bias=(expmask_sb[:, 0:1] if lt == LT - 1 else 0.0))
                    for c0, cw in ((0, 512), (512, 512), (1024, 128)):
                        psM = atps1.tile([65, 512], f32, tag="M")
                        for lt in range(LT):
                            nc.tensor.matmul(
                                psM[:, 0:cw],
                                lhsT=hp1_all[:, (b * LT + lt) * 65:(b * LT + lt + 1) * 65],
                                rhs=expT_all[:, lt * YSP + c0: lt * YSP + c0 + cw],
                                start=(lt == 0), stop=(lt == LT - 1))
                        inv_sb = at.tile([1, 512], bf16, tag="inv", bufs=2)
                        with nc.allow_low_precision(reason="softmax denom bf16"):
                            nc.vector.reciprocal(out=inv_sb[:, 0:cw],
                                                 in_=psM[64:65, 0:cw])
                        psB = atps1.tile([F, 512], f32, tag="Bc")
                        nc.tensor.matmul(psB[:, 0:cw], lhsT=ones_sb[:],
                                         rhs=inv_sb[:, 0:cw], start=True, stop=True)
                        bcast_sb = at.tile([F, 512], bf16, tag="bcast", bufs=2)
                        nc.scalar.copy(out=bcast_sb[:, 0:cw], in_=psB[:, 0:cw])
                        nc.vector.tensor_tensor(
                            out=m4tT_sb[:, b * YSP + c0: b * YSP + c0 + cw],
                            in0=psM[0:F, 0:cw], in1=bcast_sb[:, 0:cw],
                            op=mybir.AluOpType.mult)
            nc.sync.dma_start(
                out=ag2_in[:].rearrange("(p n) -> p n", p=F), in_=m4tT_sb[:])
            nc.gpsimd.collective_compute(
                "AllGather", mybir.AluOpType.bypass, replica_groups=RG,
                ins=[ag2_in.opt()], outs=[ag2_out.opt()])

            # ---------------- phase 3: graph conv + label scoring ----------------
            with tc.tile_pool(name="p3", bufs=1) as p3, \
                 tc.tile_pool(name="p3m", bufs=2) as p3m, \
                 tc.tile_pool(name="p3ps", bufs=2, space="PSUM") as p3ps:
                supp_sb = p3.tile([128, ZT * NBG], bf16)
                # m4t full, paired batches stacked on partitions for block-diag gcn
                ag2v = ag2_out.rearrange("r (f b n) -> f r b n", f=F, b=B)
                for pair in range(B // 2):
                    b0 = 2 * pair
                    mp = p3m.tile([2 * F, ZPAD], bf16, tag="mp")
                    nc.sync.dma_start(
                        out=mp[0:F, :].rearrange("p (r o n) -> p r o n", r=NC, o=1),
                        in_=ag2v[:, :, b0:b0 + 1, :])
                    nc.sync.dma_start(
                        out=mp[F:2 * F, :].rearrange("p (r o n) -> p r o n", r=NC, o=1),
                        in_=ag2v[:, :, b0 + 1:b0 + 2, :])
                    for zt in range(ZT):
                        psU = p3ps.tile([128, 128], f32, tag="U")
                        nc.tensor.matmul(psU[:, 0:2 * F],
                                         lhsT=mp[:, zt * 128:(zt + 1) * 128],
                                         rhs=gcn2_sb[:], start=True, stop=True)
                        nc.vector.tensor_copy(
                            out=supp_sb[:, zt * NBG + b0 * F: zt * NBG + (b0 + 2) * F],
                            in_=psU[:, 0:2 * F])
                for yt in range(YT):
                    psO = p3ps.tile([128, NBG], f32, tag="O")
                    for zh in range(2):
                        stripe8 = p3m.tile([128, 36 * 128], fp8, tag="adj8")
                        nc.sync.dma_start(
                            out=stripe8[:].rearrange("p (t y) -> p t y", t=36),
                            in_=adjt[yt:yt + 1, zh * 36:(zh + 1) * 36].rearrange(
                                "o t z y -> z (o t) y"))
                        if ADJ_MIXED:
                            stripe = stripe8
                        else:
                            stripe = p3m.tile([128, 36 * 128], bf16, tag="adj")
                            nc.vector.tensor_copy(out=stripe[:], in_=stripe8[:])
                        for tl in range(36):
                            zt = zh * 36 + tl
                            nc.tensor.matmul(
                                psO[:],
                                lhsT=stripe[:, tl * 128:(tl + 1) * 128],
                                rhs=supp_sb[:, zt * NBG:(zt + 1) * NBG],
                                start=(zt == 0), stop=(zt == ZT - 1))
                    o1 = p3.tile([128, NBG], f32, tag="o1", bufs=2)
                    o2 = p3.tile([128, NBG], f32, tag="o2", bufs=2)
                    nc.vector.tensor_tensor(out=o1[:], in0=psO[:], in1=gcnb_sb[:],
                                            op=mybir.AluOpType.add)
                    nc.vector.tensor_scalar_mul(o2[:], o1[:], 0.2)
                    nc.vector.tensor_tensor(out=o1[:], in0=o1[:], in1=o2[:],
                                            op=mybir.AluOpType.max)
                    stage = p3.tile([128, 16], f32, tag="stage", bufs=2)
                    scratch = p3.tile([128, F], f32, tag="scr", bufs=2)
                    tmp1 = p3.tile([128, 1], f32, tag="tmp1", bufs=2)
                    for b in range(B):
                        psT = p3ps.tile([128, 64], bf16, tag="T")
                        nc.tensor.transpose(
                            psT[:, 0:F],
                            m4tT_sb[:, b * YSP + yt * 128: b * YSP + (yt + 1) * 128],
                            identbf_sb[0:F, 0:F])
                        m4t_sb = p3.tile([128, F], bf16, tag="m4t", bufs=2)
                        nc.scalar.copy(out=m4t_sb[:], in_=psT[:, 0:F])
                        tmpa = p3.tile([128, 1], f32, tag="tmpa", bufs=2)
                        nc.vector.tensor_tensor(
                            out=scratch[:], in0=m4t_sb[:],
                            in1=f4tw_sb[:, yt * F:(yt + 1) * F],
                            op=mybir.AluOpType.mult)
                        nc.vector.reduce_sum(out=tmpa[:], in_=scratch[:],
                                             axis=mybir.AxisListType.X)
                        nc.vector.tensor_tensor(
                            out=stage[:, b:b + 1], in0=tmpa[:],
                            in1=b4t_sb[:, yt:yt + 1], op=mybir.AluOpType.add)
                        nc.vector.tensor_tensor(
                            out=scratch[:], in0=m4t_sb[:],
                            in1=f4w1_sb[:, yt * F:(yt + 1) * F],
                            op=mybir.AluOpType.mult)
                        nc.vector.reduce_sum(out=tmp1[:], in_=scratch[:],
                                             axis=mybir.AxisListType.X)
                        nc.vector.tensor_tensor(
                            out=scratch[:], in0=o1[:, b * F:(b + 1) * F],
                            in1=f4w2_sb[:, yt * F:(yt + 1) * F],
                            op=mybir.AluOpType.mult)
                        nc.vector.reduce_sum(out=tmpa[:], in_=scratch[:],
                                             axis=mybir.AxisListType.X)
                        nc.vector.tensor_tensor(
                            out=tmp1[:], in0=tmp1[:], in1=tmpa[:],
                            op=mybir.AluOpType.add)
                        nc.vector.tensor_tensor(
                            out=stage[:, 8 + b:9 + b], in0=tmp1[:],
                            in1=b4_sb[:, yt:yt + 1], op=mybir.AluOpType.add)
                    nc.sync.dma_start(out=outc[yt * 128:(yt + 1) * 128, :],
                                      in_=stage[:])

    nc.compile()
    return nc


def _bf(x):
    return np.ascontiguousarray(np.asarray(x, dtype=np.float32).astype(BF16))


def _prep_inputs(x, embed_w, conv_w, conv_b, U4_w, gcn_w, gcn_b, adj,
                 final4t_w, final4t_b, final4_w, final4_b):
    x = np.asarray(x).astype(np.int64)
    embed_w = np.asarray(embed_w, dtype=np.float32)
    conv_w = np.asarray(conv_w, dtype=np.float32)
    conv_b = np.asarray(conv_b, dtype=np.float32)
    U4_w = np.asarray(U4_w, dtype=np.float32)
    gcn_w = np.asarray(gcn_w, dtype=np.float32)
    gcn_b = np.asarray(gcn_b, dtype=np.float32)
    adj = np.asarray(adj, dtype=np.float32)
    f4t_w = np.asarray(final4t_w, dtype=np.float32)
    f4t_b = np.asarray(final4t_b, dtype=np.float32)
    f4_w = np.asarray(final4_w, dtype=np.float32)
    f4_b = np.asarray(final4_b, dtype=np.float32)

    conv_lhsT = np.zeros((E, KS * F), np.float32)
    for k in range(KS):
        conv_lhsT[:, k * F:(k + 1) * F] = conv_w[:, :, k].T
    conv_lhsT = _bf(conv_lhsT)
    conv_bias = np.ascontiguousarray(conv_b.reshape(F, 1))

    # adj scale: power of two s with s*max(adj) <= 224 (TRN fp8e4 max 240);
    # 1/s is folded into the gcn weight so the device GEMM is exact.
    amax = float(np.abs(adj).max()) or 1.0
    s = 2.0 ** np.floor(np.log2(224.0 / amax))
    gcn2 = np.zeros((2 * F, 2 * F), np.float32)
    gcn2[:F, :F] = gcn_w / s
    gcn2[F:, F:] = gcn_w / s
    gcn2 = _bf(gcn2)
    gcnb_bc = np.ascontiguousarray(
        np.broadcast_to(np.tile(gcn_b, B)[None, :], (128, NBG)))
    identbf = _bf(np.eye(128, dtype=np.float32))
    expmask = np.zeros((128, 1), np.float32)
    expmask[L - (LT - 1) * 128:, 0] = -30000.0
    ones50 = _bf(np.ones((1, F), np.float32))

    shared = dict(conv_lhsT=conv_lhsT, conv_bias=conv_bias,
                  gcn2=gcn2, gcnb_bc=gcnb_bc, identbf=identbf,
                  expmask=expmask, ones50=ones50)

    in_maps = []
    for c in range(NC):
        v = VALID[c]
        embT = np.zeros((128, LP + 8), np.float32)
        embT[:E, 4:4 + L] = embed_w[x[c]].T
        embT = _bf(embT)

        u4t_c = np.zeros((F, YSP), np.float32)
        u4t_c[:, :v] = U4_w[c * YSV:c * YSV + v].T

        at = np.zeros((ZPAD, YSP), np.float32)
        for blk in range(NC):
            vb = VALID[blk]
            at[blk * YSP:blk * YSP + vb, :v] = adj[c * YSV:c * YSV + v,
                                                   blk * YSV:blk * YSV + vb].T
        at = np.clip(at * s, 0.0, 240.0).astype(FP8)
        adjt_c = np.ascontiguousarray(
            at.reshape(ZT, 128, YT, 128).transpose(2, 0, 1, 3))

        def rowpack(w):
            out = np.zeros((128, YT * F), np.float32)
            wp = np.zeros((YSP, F), np.float32)
            wp[:v] = w[c * YSV:c * YSV + v]
            for yt in range(YT):
                out[:, yt * F:(yt + 1) * F] = wp[yt * 128:(yt + 1) * 128]
            return _bf(out)

        def biaspack(bias):
            out = np.zeros((128, YT), np.float32)
            bp = np.zeros(YSP, np.float32)
            bp[:v] = bias[c * YSV:c * YSV + v]
            out[:, :] = bp.reshape(YT, 128).T
            return np.ascontiguousarray(out)

        m = dict(shared)
        m.update(embT=embT, u4t=_bf(u4t_c), adjt=adjt_c,
                 f4tw=rowpack(f4t_w), f4w1=rowpack(f4_w[:, :F]),
                 f4w2=rowpack(f4_w[:, F:]), b4t=biaspack(f4t_b),
                 b4=biaspack(f4_b))
        in_maps.append(m)
    return in_maps


def _postprocess(results):
    y4t = np.zeros((B, Y), np.float32)
    y4 = np.zeros((B, Y), np.float32)
    for c in range(NC):
        v = VALID[c]
        oc = results[c]["outc"]
        y4t[:, c * YSV:c * YSV + v] = oc[:v, 0:8].T
        y4[:, c * YSV:c * YSV + v] = oc[:v, 8:16].T
    return y4t, y4


def _get_nc():
    if "nc" not in _CACHE:
        _CACHE["nc"] = _build()
    return _CACHE["nc"]


def run_raw(in_maps, **kw):
    nc = _get_nc()
    return run_bass_kernel_spmd(nc, in_maps, list(range(NC)), **kw)


def kernel(x, target, embed_w, conv_w, conv_b, U4_w, gcn_w, gcn_b, adj,
           final4t_w, final4t_b, final4_w, final4_b):
    in_maps = _prep_inputs(x, embed_w, conv_w, conv_b, U4_w, gcn_w, gcn_b, adj,
                           final4t_w, final4t_b, final4_w, final4_b)
    res = run_raw(in_maps)
    return _postprocess(res.results)


# revision 20
# speedup vs baseline: 8980.9444x; 1.1997x over previous
"""Trainium2 Bass kernel for nn_ConvAttnPool (conv + per-label attention pooling
+ label-graph conv + label-wise scoring), SPMD over 8 NeuronCores.

Sharding: label dim Y=8922 is split 8 ways (1116/core, padded to 1152); the
front-end conv is sharded over batch (1 batch/core) followed by an AllGather of
the conv activations; a second AllGather exchanges the per-label pooled
features (m4t) for the graph conv.

Host-side prep keeps the per-call device payload small (the axon PJRT tunnel
is ~70 MB/s): the embedding lookup is resolved on host (ship the 0.66 MB
gathered+transposed activations instead of the 25.6 MB vocab table per core),
and the adjacency is pre-transposed, stripe-contiguous (one DMA descriptor per
partition), scaled by a power of two s and stored fp8e4 (TRN E4M3, max 240) —
the matching 1/s is folded into the gcn weight so the device computes exactly
adj @ support with no extra ops.
"""
import numpy as np
import ml_dtypes

import concourse.bass as bass
import concourse.bacc as bacc
import concourse.tile as tile
from concourse import mybir
from concourse.bass_utils import run_bass_kernel_spmd

BF16 = ml_dtypes.bfloat16
FP8 = ml_dtypes.float8_e4m3

# problem dims (hardcoded per contract)
B, L, V, E, F, KS, Y = 8, 2500, 50002, 100, 50, 9, 8922
NC = 8
YSV = 1116                   # labels per core (last core has 1110 valid)
YSP = 1152                   # padded labels per core
YT = YSP // 128              # 9 y-tiles
LP = 2560                    # padded seq len
LT = LP // 128               # 20 l-tiles
ZPAD = NC * YSP              # 9216 padded global label dim
ZT = ZPAD // 128             # 72 z-tiles
NBG = B * F                  # 400
VALID = [YSV] * (NC - 1) + [Y - (NC - 1) * YSV]

f32 = mybir.dt.float32
bf16 = mybir.dt.bfloat16
fp8 = mybir.dt.float8e4
i32 = mybir.dt.int32

_CACHE = {}
import os
ADJ_MIXED = int(os.environ.get('K_ADJ_MIXED', '1'))
DBG = int(os.environ.get('K_DBG', '0'))


def _build():
    nc = bacc.Bacc("TRN2", target_bir_lowering=False, debug=False,
                   enable_asserts=True, num_devices=NC)

    embT = nc.dram_tensor("embT", [128, LP + 8], bf16, kind="ExternalInput")
    conv_lhsT = nc.dram_tensor("conv_lhsT", [E, KS * F], bf16, kind="ExternalInput")
    conv_bias = nc.dram_tensor("conv_bias", [F, 1], f32, kind="ExternalInput")
    u4t = nc.dram_tensor("u4t", [F, YSP], bf16, kind="ExternalInput")
    adjt = nc.dram_tensor("adjt", [YT, 2, 128, 36 * 128], fp8, kind="ExternalInput")
    gcn2 = nc.dram_tensor("gcn2", [2 * F, 2 * F], bf16, kind="ExternalInput")
    gcnb_bc = nc.dram_tensor("gcnb_bc", [128, NBG], f32, kind="ExternalInput")
    f4tw = nc.dram_tensor("f4tw", [128, YT * F], bf16, kind="ExternalInput")
    f4w1 = nc.dram_tensor("f4w1", [128, YT * F], bf16, kind="ExternalInput")
    f4w2 = nc.dram_tensor("f4w2", [128, YT * F], bf16, kind="ExternalInput")
    b4t = nc.dram_tensor("b4t", [128, YT], f32, kind="ExternalInput")
    b4 = nc.dram_tensor("b4", [128, YT], f32, kind="ExternalInput")
    identbf = nc.dram_tensor("identbf", [128, 128], bf16, kind="ExternalInput")
    expmask = nc.dram_tensor("expmask", [128, 1], f32, kind="ExternalInput")
    ones50 = nc.dram_tensor("ones50", [1, F], f32, kind="ExternalInput")
    outc = nc.dram_tensor("outc", [YSP, 16], f32, kind="ExternalOutput")
    if DBG:
        dbg_m4t = nc.dram_tensor("dbg_m4t", [F, B * YSP], bf16, kind="ExternalOutput")
        dbg_m4a = nc.dram_tensor("dbg_m4a", [128, NBG], bf16, kind="ExternalOutput")
        dbg_f4 = nc.dram_tensor("dbg_f4", [128, YT * NBG], bf16, kind="ExternalOutput")
        dbg_supp = nc.dram_tensor("dbg_supp", [128, ZT * NBG], bf16, kind="ExternalOutput")
        dbg_hp1 = nc.dram_tensor("dbg_hp1", [128, LT * 65], bf16, kind="ExternalOutput")
        dbg_embT = nc.dram_tensor("dbg_embT", [128, LP + 8], bf16, kind="ExternalOutput")
        dbg_hpT = nc.dram_tensor("dbg_hpT", [F, LP], bf16, kind="ExternalOutput")

    Exp = mybir.ActivationFunctionType.Exp
    Tanh = mybir.ActivationFunctionType.Tanh
    RG = [list(range(NC))]
    Mult = mybir.AluOpType.mult
    Add = mybir.AluOpType.add
    Max = mybir.AluOpType.max

    with tile.TileContext(nc) as tc:
        with tc.tile_pool(name="const", bufs=1) as cp, \
             tc.tile_pool(name="pers", bufs=1) as pers, \
             tc.tile_pool(name="dram", bufs=1, space="DRAM") as dram:
            identbf_sb = cp.tile([128, 128], bf16)
            nc.sync.dma_start(out=identbf_sb[:], in_=identbf[:])
            convw_sb = cp.tile([E, KS * F], bf16)
            nc.sync.dma_start(out=convw_sb[:], in_=conv_lhsT[:])
            convb_sb = cp.tile([F, 1], f32)
            nc.sync.dma_start(out=convb_sb[:], in_=conv_bias[:])
            u4t_sb = cp.tile([F, YSP], bf16)
            nc.sync.dma_start(out=u4t_sb[:], in_=u4t[:])
            gcn2_sb = cp.tile([2 * F, 2 * F], bf16)
            nc.sync.dma_start(out=gcn2_sb[:], in_=gcn2[:])
            gcnb_sb = cp.tile([128, NBG], f32)
            nc.sync.dma_start(out=gcnb_sb[:], in_=gcnb_bc[:])
            f4tw_sb = cp.tile([128, YT * F], bf16)
            nc.sync.dma_start(out=f4tw_sb[:], in_=f4tw[:])
            f4w1_sb = cp.tile([128, YT * F], bf16)
            nc.sync.dma_start(out=f4w1_sb[:], in_=f4w1[:])
            f4w2_sb = cp.tile([128, YT * F], bf16)
            nc.sync.dma_start(out=f4w2_sb[:], in_=f4w2[:])
            b4t_sb = cp.tile([128, YT], f32)
            nc.sync.dma_start(out=b4t_sb[:], in_=b4t[:])
            b4_sb = cp.tile([128, YT], f32)
            nc.sync.dma_start(out=b4_sb[:], in_=b4[:])
            expmask_sb = cp.tile([128, 1], f32)
            nc.sync.dma_start(out=expmask_sb[:], in_=expmask[:])
            ones_sb = cp.tile([1, F], f32)
            nc.sync.dma_start(out=ones_sb[:], in_=ones50[:])

            m4tT_sb = pers.tile([F, B * YSP], bf16)   # this core's label slice

            HPT_SZ = F * LP               # 128000
            AG1N = HPT_SZ + 128 * LT * 65
            ag1_in = dram.tile([AG1N], bf16)
            ag1_out = dram.tile([NC, AG1N], bf16, addr_space="Shared")

            # ---------------- phase 1: conv on own batch ----------------
            with tc.tile_pool(name="p1", bufs=1) as p1, \
                 tc.tile_pool(name="p1ps", bufs=2, space="PSUM") as p1ps:
                embT_sb = p1.tile([128, LP + 8], bf16)
                nc.sync.dma_start(out=embT_sb[:], in_=embT[:])
                hpT_sb = p1.tile([F, LP], bf16)
                for l5 in range(5):
                    psc = p1ps.tile([F, 512], f32, tag="conv")
                    for k in range(KS):
                        nc.tensor.matmul(
                            psc[:],
                            lhsT=convw_sb[:, k * F:(k + 1) * F],
                            rhs=embT_sb[0:E, l5 * 512 + k: l5 * 512 + k + 512],
                            start=(k == 0), stop=(k == KS - 1))
                    nc.scalar.activation(out=hpT_sb[:, l5 * 512:(l5 + 1) * 512],
                                         in_=psc[:], func=Tanh,
                                         bias=convb_sb[:, 0:1])
                hp1_sb = p1.tile([128, LT * 65], bf16)
                nc.vector.memset(hp1_sb[:], 1.0)
                for lt in range(LT):
                    pst2 = p1ps.tile([128, 64], bf16, tag="tp2")
                    nc.tensor.transpose(pst2[:, 0:F],
                                        hpT_sb[:, lt * 128:(lt + 1) * 128],
                                        identbf_sb[0:F, 0:F])
                    nc.scalar.copy(out=hp1_sb[:, lt * 65:lt * 65 + F],
                                   in_=pst2[:, 0:F])
                if DBG:
                    nc.sync.dma_start(out=dbg_hp1[:], in_=hp1_sb[:])
                    nc.sync.dma_start(out=dbg_embT[:], in_=embT_sb[:])
                    nc.sync.dma_start(out=dbg_hpT[:], in_=hpT_sb[:])
                nc.sync.dma_start(
                    out=ag1_in[0:HPT_SZ].rearrange("(p n) -> p n", p=F),
                    in_=hpT_sb[:])
                nc.sync.dma_start(
                    out=ag1_in[HPT_SZ:AG1N].rearrange("(p n) -> p n", p=128),
                    in_=hp1_sb[:])
            nc.gpsimd.collective_compute(
                "AllGather", mybir.AluOpType.bypass, replica_groups=RG,
                ins=[ag1_in.opt()], outs=[ag1_out.opt()])

            ag2_in = dram.tile([F * B * YSP], bf16)
            ag2_out = dram.tile([NC, F * B * YSP], bf16, addr_space="Shared")

            # ---------------- phase 2: per-label attention ----------------
            with tc.tile_pool(name="attn", bufs=1) as at, \
                 tc.tile_pool(name="atps", bufs=2, space="PSUM") as atps, \
                 tc.tile_pool(name="atps1", bufs=1, space="PSUM") as atps1:
                hpT_all = at.tile([F, NC * LP], bf16)
                hp1_all = at.tile([128, NC * LT * 65], bf16)
                for r in range(NC):
                    nc.sync.dma_start(
                        out=hpT_all[:, r * LP:(r + 1) * LP],
                        in_=ag1_out[r:r + 1, 0:HPT_SZ].rearrange(
                            "o (p n) -> (o p) n", p=F))
                    nc.sync.dma_start(
                        out=hp1_all[:, r * LT * 65:(r + 1) * LT * 65],
                        in_=ag1_out[r:r + 1, HPT_SZ:AG1N].rearrange(
                            "o (p n) -> (o p) n", p=128))
                for b in range(B):
                    expT_all = at.tile([128, LT * YSP], bf16, tag="expT", bufs=2)
                    for lt in range(LT):
                        psS = atps.tile([128, YSP], f32, tag="S")
                        for c0, cw in ((0, 512), (512, 512), (1024, 128)):
                            nc.tensor.matmul(
                                psS[:, c0:c0 + cw],
                                lhsT=hpT_all[:, b * LP + lt * 128: b * LP + (lt + 1) * 128],
                                rhs=u4t_sb[:, c0:c0 + cw],
                                start=True, stop=True)
                        nc.scalar.activation(
                            out=expT_all[:, lt * YSP:(lt + 1) * YSP],
                            in_=psS[:], func=Exp,
                            bias=(expmask_sb[:, 0:1] if lt == LT - 1 else 0.0))
                    for c0, cw in ((0, 512), (512, 512), (1024, 128)):
                        psM = atps1.tile([65, 512], f32, tag="M")
                        for lt in range(LT):
                            nc.tensor.matmul(
                                psM[:, 0:cw],
                                lhsT=hp1_all[:, (b * LT + lt) * 65:(b * LT + lt + 1) * 65],
                                rhs=expT_all[:, lt * YSP + c0: lt * YSP + c0 + cw],
                                start=(lt == 0), stop=(lt == LT - 1))
                        inv32 = at.tile([1, 512], f32, tag="inv32", bufs=2)
                        nc.vector.reciprocal(
                            out=inv32[:, 0:cw], in_=psM[64:65, 0:cw])
                        psB = atps1.tile([F, 512], f32, tag="Bc")
                        nc.tensor.matmul(psB[:, 0:cw], lhsT=ones_sb[:],
                                         rhs=inv32[:, 0:cw], start=True, stop=True)
                        bcast_sb = at.tile([F, 512], bf16, tag="bcast", bufs=2)
                        nc.scalar.copy(out=bcast_sb[:, 0:cw], in_=psB[:, 0:cw])
                        nc.vector.tensor_tensor(
                            out=m4tT_sb[:, b * YSP + c0: b * YSP + c0 + cw],
                            in0=psM[0:F, 0:cw], in1=bcast_sb[:, 0:cw],
                            op=Mult)
            if DBG:
                nc.sync.dma_start(out=dbg_m4t[:], in_=m4tT_sb[:])
            nc.sync.dma_start(
                out=ag2_in[:].rearrange("(p n) -> p n", p=F), in_=m4tT_sb[:])
            nc.gpsimd.collective_compute(
                "AllGather", mybir.AluOpType.bypass, replica_groups=RG,
                ins=[ag2_in.opt()], outs=[ag2_out.opt()])

            # ---------------- phase 3: graph conv + label scoring ----------------
            with tc.tile_pool(name="p3", bufs=1) as p3, \
                 tc.tile_pool(name="p3m", bufs=2) as p3m, \
                 tc.tile_pool(name="p3ps", bufs=2, space="PSUM") as p3ps:
                # final-layer weights repeated per batch slot (done once, on-chip)
                f4tw8 = p3.tile([128, YT * NBG], bf16)
                f4w18 = p3.tile([128, YT * NBG], bf16)
                f4w28 = p3.tile([128, YT * NBG], bf16)
                for dst, src in ((f4tw8, f4tw_sb), (f4w18, f4w1_sb), (f4w28, f4w2_sb)):
                    dv = dst[:].rearrange("p (t b f) -> p t b f", t=YT, b=B)
                    sv = src[:].rearrange("p (t f) -> p t f", t=YT)
                    for b in range(B):
                        nc.scalar.copy(out=dv[:, :, b, :], in_=sv)

                if DBG:
                    nc.sync.dma_start(out=dbg_f4[:], in_=f4tw8[:])
                supp_sb = p3.tile([128, ZT * NBG], bf16)
                # m4t full, paired batches stacked on partitions for block-diag gcn
                ag2v = ag2_out.rearrange("r (f b n) -> f r b n", f=F, b=B)
                with tc.tile_pool(name="p3mp", bufs=1) as p3mp:
                    mpall = p3mp.tile([2 * F, 4 * ZPAD], bf16)
                    for pair in range(B // 2):
                        b0 = 2 * pair
                        nc.sync.dma_start(
                            out=mpall[0:F, pair * ZPAD:(pair + 1) * ZPAD].rearrange(
                                "p (r o n) -> p r o n", r=NC, o=1),
                            in_=ag2v[:, :, b0:b0 + 1, :])
                        nc.sync.dma_start(
                            out=mpall[F:2 * F, pair * ZPAD:(pair + 1) * ZPAD].rearrange(
                                "p (r o n) -> p r o n", r=NC, o=1),
                            in_=ag2v[:, :, b0 + 1:b0 + 2, :])
                    for zt in range(ZT):
                        psU = p3ps.tile([128, NBG], f32, tag="U")
                        for pair in range(B // 2):
                            nc.tensor.matmul(
                                psU[:, pair * 100:(pair + 1) * 100],
                                lhsT=mpall[:, pair * ZPAD + zt * 128:
                                           pair * ZPAD + (zt + 1) * 128],
                                rhs=gcn2_sb[:], start=True, stop=True)
                        nc.vector.tensor_copy(
                            out=supp_sb[:, zt * NBG:(zt + 1) * NBG], in_=psU[:])

                if DBG:
                    nc.sync.dma_start(out=dbg_supp[:], in_=supp_sb[:])
                for yt in range(YT):
                    psO = p3ps.tile([128, NBG], f32, tag="O")
                    for zh in range(2):
                        stripe8 = p3m.tile([128, 36 * 128], fp8, tag="adj8")
                        nc.sync.dma_start(
                            out=stripe8[:],
                            in_=adjt[yt:yt + 1, zh:zh + 1].rearrange(
                                "o q p n -> (o q p) n"))
                        if ADJ_MIXED:
                            stripe = stripe8
                        else:
                            stripe = p3m.tile([128, 36 * 128], bf16, tag="adj")
                            nc.vector.tensor_copy(out=stripe[:], in_=stripe8[:])
                        for tl in range(36):
                            zt = zh * 36 + tl
                            nc.tensor.matmul(
                                psO[:],
                                lhsT=stripe[:, tl * 128:(tl + 1) * 128],
                                rhs=supp_sb[:, zt * NBG:(zt + 1) * NBG],
                                start=(zt == 0), stop=(zt == ZT - 1))
                    o1 = p3.tile([128, NBG], f32, tag="o1", bufs=2)
                    o2 = p3.tile([128, NBG], f32, tag="o2", bufs=2)
                    nc.vector.tensor_tensor(out=o1[:], in0=psO[:], in1=gcnb_sb[:],
                                            op=Add)
                    nc.vector.tensor_scalar_mul(o2[:], o1[:], 0.2)
                    nc.vector.tensor_tensor(out=o1[:], in0=o1[:], in1=o2[:], op=Max)
                    # batched per-label scoring over all 8 batches at once
                    m4t_all = p3.tile([128, NBG], bf16, tag="m4a", bufs=2)
                    for b in range(B):
                        psT = p3ps.tile([128, 64], bf16, tag="T")
                        nc.tensor.transpose(
                            psT[:, 0:F],
                            m4tT_sb[:, b * YSP + yt * 128: b * YSP + (yt + 1) * 128],
                            identbf_sb[0:F, 0:F])
                        nc.scalar.copy(out=m4t_all[:, b * F:(b + 1) * F],
                                       in_=psT[:, 0:F])
                    scr = p3.tile([128, NBG], f32, tag="scr", bufs=2)
                    acc1 = p3.tile([128, B], f32, tag="a1", bufs=2)
                    acc2 = p3.tile([128, B], f32, tag="a2", bufs=2)
                    stage = p3.tile([128, 16], f32, tag="stage", bufs=2)
                    nc.vector.tensor_tensor(
                        out=scr[:], in0=m4t_all[:],
                        in1=f4tw8[:, yt * NBG:(yt + 1) * NBG], op=Mult)
                    nc.vector.reduce_sum(
                        out=acc1[:], in_=scr[:].rearrange("p (b f) -> p b f", b=B),
                        axis=mybir.AxisListType.X)
                    nc.vector.tensor_scalar(
                        out=stage[:, 0:8], in0=acc1[:],
                        scalar1=b4t_sb[:, yt:yt + 1], scalar2=None, op0=Add)
                    nc.vector.tensor_tensor(
                        out=scr[:], in0=m4t_all[:],
                        in1=f4w18[:, yt * NBG:(yt + 1) * NBG], op=Mult)
                    nc.vector.reduce_sum(
                        out=acc1[:], in_=scr[:].rearrange("p (b f) -> p b f", b=B),
                        axis=mybir.AxisListType.X)
                    nc.vector.tensor_tensor(
                        out=scr[:], in0=o1[:],
                        in1=f4w28[:, yt * NBG:(yt + 1) * NBG], op=Mult)
                    nc.vector.reduce_sum(
                        out=acc2[:], in_=scr[:].rearrange("p (b f) -> p b f", b=B),
                        axis=mybir.AxisListType.X)
                    nc.vector.tensor_tensor(out=acc1[:], in0=acc1[:], in1=acc2[:],
                                            op=Add)
                    nc.vector.tensor_scalar(
                        out=stage[:, 8:16], in0=acc1[:],
                        scalar1=b4_sb[:, yt:yt + 1], scalar2=None, op0=Add)
                    if DBG and yt == 0:
                        nc.sync.dma_start(out=dbg_m4a[:], in_=m4t_all[:])
                    nc.sync.dma_start(out=outc[yt * 128:(yt + 1) * 128, :],
                                      in_=stage[:])

    nc.compile()
    return nc


def _bf(x):
    return np.ascontiguousarray(np.asarray(x, dtype=np.float32).astype(BF16))


def _prep_inputs(x, embed_w, conv_w, conv_b, U4_w, gcn_w, gcn_b, adj,
                 final4t_w, final4t_b, final4_w, final4_b):
    x = np.asarray(x).astype(np.int64)
    embed_w = np.asarray(embed_w, dtype=np.float32)
    conv_w = np.asarray(conv_w, dtype=np.float32)
    conv_b = np.asarray(conv_b, dtype=np.float32)
    U4_w = np.asarray(U4_w, dtype=np.float32)
    gcn_w = np.asarray(gcn_w, dtype=np.float32)
    gcn_b = np.asarray(gcn_b, dtype=np.float32)
    adj = np.asarray(adj, dtype=np.float32)
    f4t_w = np.asarray(final4t_w, dtype=np.float32)
    f4t_b = np.asarray(final4t_b, dtype=np.float32)
    f4_w = np.asarray(final4_w, dtype=np.float32)
    f4_b = np.asarray(final4_b, dtype=np.float32)

    conv_lhsT = np.zeros((E, KS * F), np.float32)
    for k in range(KS):
        conv_lhsT[:, k * F:(k + 1) * F] = conv_w[:, :, k].T
    conv_lhsT = _bf(conv_lhsT)
    conv_bias = np.ascontiguousarray(conv_b.reshape(F, 1))

    # adj scale: power of two s with s*max(adj) <= 224 (TRN fp8e4 max 240);
    # 1/s is folded into the gcn weight so the device GEMM is exact.
    amax = float(np.abs(adj).max()) or 1.0
    s = 2.0 ** np.floor(np.log2(224.0 / amax))
    gcn2 = np.zeros((2 * F, 2 * F), np.float32)
    gcn2[:F, :F] = gcn_w / s
    gcn2[F:, F:] = gcn_w / s
    gcn2 = _bf(gcn2)
    gcnb_bc = np.ascontiguousarray(
        np.broadcast_to(np.tile(gcn_b, B)[None, :], (128, NBG)))
    identbf = _bf(np.eye(128, dtype=np.float32))
    expmask = np.zeros((128, 1), np.float32)
    expmask[L - (LT - 1) * 128:, 0] = -30000.0
    ones50 = np.ones((1, F), np.float32)

    shared = dict(conv_lhsT=conv_lhsT, conv_bias=conv_bias,
                  gcn2=gcn2, gcnb_bc=gcnb_bc, identbf=identbf,
                  expmask=expmask, ones50=ones50)

    in_maps = []
    for c in range(NC):
        v = VALID[c]
        embT = np.zeros((128, LP + 8), np.float32)
        embT[:E, 4:4 + L] = embed_w[x[c]].T
        embT = _bf(embT)

        u4t_c = np.zeros((F, YSP), np.float32)
        u4t_c[:, :v] = U4_w[c * YSV:c * YSV + v].T

        at = np.zeros((ZPAD, YSP), np.float32)
        for blk in range(NC):
            vb = VALID[blk]
            at[blk * YSP:blk * YSP + vb, :v] = adj[c * YSV:c * YSV + v,
                                                   blk * YSV:blk * YSV + vb].T
        at = np.clip(at * s, 0.0, 240.0).astype(FP8)
        # stripe-contiguous: [yt, zh, zp, t, y] so each [128, 36*128] stripe
        # DMA is one 4608B descriptor per partition
        adjt_c = np.ascontiguousarray(
            at.reshape(2, 36, 128, YT, 128).transpose(3, 0, 2, 1, 4)
            .reshape(YT, 2, 128, 36 * 128))

        def rowpack(w):
            out = np.zeros((128, YT * F), np.float32)
            wp = np.zeros((YSP, F), np.float32)
            wp[:v] = w[c * YSV:c * YSV + v]
            for yt in range(YT):
                out[:, yt * F:(yt + 1) * F] = wp[yt * 128:(yt + 1) * 128]
            return _bf(out)

        def biaspack(bias):
            out = np.zeros((128, YT), np.float32)
            bp = np.zeros(YSP, np.float32)
            bp[:v] = bias[c * YSV:c * YSV + v]
            out[:, :] = bp.reshape(YT, 128).T
            return np.ascontiguousarray(out)

        m = dict(shared)
        m.update(embT=embT, u4t=_bf(u4t_c), adjt=adjt_c,
                 f4tw=rowpack(f4t_w), f4w1=rowpack(f4_w[:, :F]),
                 f4w2=rowpack(f4_w[:, F:]), b4t=biaspack(f4t_b),
                 b4=biaspack(f4_b))
        in_maps.append(m)
    return in_maps


def _postprocess(results):
    y4t = np.zeros((B, Y), np.float32)
    y4 = np.zeros((B, Y), np.float32)
    for c in range(NC):
        v = VALID[c]
        oc = results[c]["outc"]
        y4t[:, c * YSV:c * YSV + v] = oc[:v, 0:8].T
        y4[:, c * YSV:c * YSV + v] = oc[:v, 8:16].T
    return y4t, y4


def _get_nc():
    if "nc" not in _CACHE:
        _CACHE["nc"] = _build()
    return _CACHE["nc"]


def run_raw(in_maps, **kw):
    nc = _get_nc()
    return run_bass_kernel_spmd(nc, in_maps, list(range(NC)), **kw)


def kernel(x, target, embed_w, conv_w, conv_b, U4_w, gcn_w, gcn_b, adj,
           final4t_w, final4t_b, final4_w, final4_b):
    in_maps = _prep_inputs(x, embed_w, conv_w, conv_b, U4_w, gcn_w, gcn_b, adj,
                           final4t_w, final4t_b, final4_w, final4_b)
    res = run_raw(in_maps)
    return _postprocess(res.results)


# revision 21
# speedup vs baseline: 10027.2966x; 1.1165x over previous
"""Trainium2 Bass kernel for nn_ConvAttnPool (conv + per-label attention pooling
+ label-graph conv + label-wise scoring), SPMD over 8 NeuronCores.

Sharding: label dim Y=8922 is split 8 ways (1116/core, padded to 1152); the
front-end conv is sharded over batch (1 batch/core) followed by an AllGather of
the conv activations; a second AllGather exchanges the per-label pooled
features (m4t) for the graph conv.

Host-side prep keeps the per-call device payload small (the axon PJRT tunnel
is ~70 MB/s): the embedding lookup is resolved on host (ship the 0.66 MB
gathered+transposed activations instead of the 25.6 MB vocab table per core),
and the adjacency is pre-transposed, stripe-contiguous (one DMA descriptor per
partition), scaled by a power of two s and stored fp8e4 (TRN E4M3, max 240) —
the matching 1/s is folded into the gcn weight so the device computes exactly
adj @ support with no extra ops.
"""
import numpy as np
import ml_dtypes

import concourse.bass as bass
import concourse.bacc as bacc
import concourse.tile as tile
from concourse import mybir
from concourse.bass_utils import run_bass_kernel_spmd

BF16 = ml_dtypes.bfloat16
FP8 = ml_dtypes.float8_e4m3

# problem dims (hardcoded per contract)
B, L, V, E, F, KS, Y = 8, 2500, 50002, 100, 50, 9, 8922
NC = 8
YSV = 1116                   # labels per core (last core has 1110 valid)
YSP = 1152                   # padded labels per core
YT = YSP // 128              # 9 y-tiles
LP = 2560                    # padded seq len
LT = LP // 128               # 20 l-tiles
ZPAD = NC * YSP              # 9216 padded global label dim
ZT = ZPAD // 128             # 72 z-tiles
NBG = B * F                  # 400
VALID = [YSV] * (NC - 1) + [Y - (NC - 1) * YSV]

f32 = mybir.dt.float32
bf16 = mybir.dt.bfloat16
fp8 = mybir.dt.float8e4
i32 = mybir.dt.int32

_CACHE = {}
import os
ADJ_MIXED = int(os.environ.get('K_ADJ_MIXED', '1'))
DR = int(os.environ.get('K_DR', '1'))
DBG = int(os.environ.get('K_DBG', '0'))


def _build():
    nc = bacc.Bacc("TRN2", target_bir_lowering=False, debug=False,
                   enable_asserts=True, num_devices=NC)

    embT = nc.dram_tensor("embT", [128, LP + 8], bf16, kind="ExternalInput")
    conv_lhsT = nc.dram_tensor("conv_lhsT", [E, KS * F], bf16, kind="ExternalInput")
    conv_bias = nc.dram_tensor("conv_bias", [F, 1], f32, kind="ExternalInput")
    u4t = nc.dram_tensor("u4t", [F, YSP], bf16, kind="ExternalInput")
    adjt = nc.dram_tensor("adjt", [YT, 2, 128, 36 * 128], fp8, kind="ExternalInput")
    gcn2 = nc.dram_tensor("gcn2", [2 * F, 2 * F], bf16, kind="ExternalInput")
    gcnb_bc = nc.dram_tensor("gcnb_bc", [128, NBG], f32, kind="ExternalInput")
    f4tw = nc.dram_tensor("f4tw", [128, YT * F], bf16, kind="ExternalInput")
    f4w1 = nc.dram_tensor("f4w1", [128, YT * F], bf16, kind="ExternalInput")
    f4w2 = nc.dram_tensor("f4w2", [128, YT * F], bf16, kind="ExternalInput")
    b4t = nc.dram_tensor("b4t", [128, YT], f32, kind="ExternalInput")
    b4 = nc.dram_tensor("b4", [128, YT], f32, kind="ExternalInput")
    identbf = nc.dram_tensor("identbf", [128, 128], bf16, kind="ExternalInput")
    expmask = nc.dram_tensor("expmask", [128, 1], f32, kind="ExternalInput")
    ones50 = nc.dram_tensor("ones50", [1, F], bf16, kind="ExternalInput")
    outc = nc.dram_tensor("outc", [YSP, 16], f32, kind="ExternalOutput")
    if DBG:
        dbg_m4t = nc.dram_tensor("dbg_m4t", [F, B * YSP], bf16, kind="ExternalOutput")
        dbg_m4a = nc.dram_tensor("dbg_m4a", [128, NBG], bf16, kind="ExternalOutput")
        dbg_f4 = nc.dram_tensor("dbg_f4", [128, YT * NBG], bf16, kind="ExternalOutput")
        dbg_supp = nc.dram_tensor("dbg_supp", [128, ZT * NBG], fp8, kind="ExternalOutput")
        dbg_hp1 = nc.dram_tensor("dbg_hp1", [128, LT * 65], bf16, kind="ExternalOutput")
        dbg_embT = nc.dram_tensor("dbg_embT", [128, LP + 8], bf16, kind="ExternalOutput")
        dbg_hpT = nc.dram_tensor("dbg_hpT", [F, LP], bf16, kind="ExternalOutput")

    Exp = mybir.ActivationFunctionType.Exp
    Tanh = mybir.ActivationFunctionType.Tanh
    RG = [list(range(NC))]
    Mult = mybir.AluOpType.mult
    Add = mybir.AluOpType.add
    Max = mybir.AluOpType.max

    with tile.TileContext(nc) as tc:
        with tc.tile_pool(name="const", bufs=1) as cp, \
             tc.tile_pool(name="pers", bufs=1) as pers, \
             tc.tile_pool(name="dram", bufs=1, space="DRAM") as dram:
            identbf_sb = cp.tile([128, 128], bf16)
            nc.sync.dma_start(out=identbf_sb[:], in_=identbf[:])
            convw_sb = cp.tile([E, KS * F], bf16)
            nc.sync.dma_start(out=convw_sb[:], in_=conv_lhsT[:])
            convb_sb = cp.tile([F, 1], f32)
            nc.sync.dma_start(out=convb_sb[:], in_=conv_bias[:])
            u4t_sb = cp.tile([F, YSP], bf16)
            nc.sync.dma_start(out=u4t_sb[:], in_=u4t[:])
            gcn2_sb = cp.tile([2 * F, 2 * F], bf16)
            nc.sync.dma_start(out=gcn2_sb[:], in_=gcn2[:])
            gcnb_sb = cp.tile([128, NBG], f32)
            nc.sync.dma_start(out=gcnb_sb[:], in_=gcnb_bc[:])
            f4tw_sb = cp.tile([128, YT * F], bf16)
            nc.sync.dma_start(out=f4tw_sb[:], in_=f4tw[:])
            f4w1_sb = cp.tile([128, YT * F], bf16)
            nc.sync.dma_start(out=f4w1_sb[:], in_=f4w1[:])
            f4w2_sb = cp.tile([128, YT * F], bf16)
            nc.sync.dma_start(out=f4w2_sb[:], in_=f4w2[:])
            b4t_sb = cp.tile([128, YT], f32)
            nc.sync.dma_start(out=b4t_sb[:], in_=b4t[:])
            b4_sb = cp.tile([128, YT], f32)
            nc.sync.dma_start(out=b4_sb[:], in_=b4[:])
            expmask_sb = cp.tile([128, 1], f32)
            nc.sync.dma_start(out=expmask_sb[:], in_=expmask[:])
            ones_sb = cp.tile([1, F], bf16)
            nc.sync.dma_start(out=ones_sb[:], in_=ones50[:])

            m4tT_sb = pers.tile([F, B * YSP], bf16)   # this core's label slice
            m4tf8_sb = pers.tile([F, B * YSP], fp8)   # fp8 twin for the gcn path

            HPT_SZ = F * LP               # 128000
            AG1N = HPT_SZ + 128 * LT * 65
            ag1_in = dram.tile([AG1N], bf16)
            ag1_out = dram.tile([NC, AG1N], bf16, addr_space="Shared")

            # ---------------- phase 1: conv on own batch ----------------
            with tc.tile_pool(name="p1", bufs=1) as p1, \
                 tc.tile_pool(name="p1ps", bufs=2, space="PSUM") as p1ps:
                embT_sb = p1.tile([128, LP + 8], bf16)
                nc.sync.dma_start(out=embT_sb[:], in_=embT[:])
                hpT_sb = p1.tile([F, LP], bf16)
                for l5 in range(5):
                    psc = p1ps.tile([F, 512], f32, tag="conv")
                    for k in range(KS):
                        nc.tensor.matmul(
                            psc[:],
                            lhsT=convw_sb[:, k * F:(k + 1) * F],
                            rhs=embT_sb[0:E, l5 * 512 + k: l5 * 512 + k + 512],
                            start=(k == 0), stop=(k == KS - 1))
                    nc.scalar.activation(out=hpT_sb[:, l5 * 512:(l5 + 1) * 512],
                                         in_=psc[:], func=Tanh,
                                         bias=convb_sb[:, 0:1])
                hp1_sb = p1.tile([128, LT * 65], bf16)
                nc.vector.memset(hp1_sb[:], 1.0)
                for lt in range(LT):
                    pst2 = p1ps.tile([128, 64], bf16, tag="tp2")
                    nc.tensor.transpose(pst2[:, 0:F],
                                        hpT_sb[:, lt * 128:(lt + 1) * 128],
                                        identbf_sb[0:F, 0:F])
                    nc.scalar.copy(out=hp1_sb[:, lt * 65:lt * 65 + F],
                                   in_=pst2[:, 0:F])
                if DBG:
                    nc.sync.dma_start(out=dbg_hp1[:], in_=hp1_sb[:])
                    nc.sync.dma_start(out=dbg_embT[:], in_=embT_sb[:])
                    nc.sync.dma_start(out=dbg_hpT[:], in_=hpT_sb[:])
                nc.sync.dma_start(
                    out=ag1_in[0:HPT_SZ].rearrange("(p n) -> p n", p=F),
                    in_=hpT_sb[:])
                nc.sync.dma_start(
                    out=ag1_in[HPT_SZ:AG1N].rearrange("(p n) -> p n", p=128),
                    in_=hp1_sb[:])
            nc.gpsimd.collective_compute(
                "AllGather", mybir.AluOpType.bypass, replica_groups=RG,
                ins=[ag1_in.opt()], outs=[ag1_out.opt()])

            ag2_in = dram.tile([F * B * YSP], fp8)
            ag2_out = dram.tile([NC, F * B * YSP], fp8, addr_space="Shared")

            # ---------------- phase 2: per-label attention ----------------
            with tc.tile_pool(name="attn", bufs=1) as at, \
                 tc.tile_pool(name="atps", bufs=2, space="PSUM") as atps, \
                 tc.tile_pool(name="atps1", bufs=1, space="PSUM") as atps1:
                hpT_all = at.tile([F, NC * LP], bf16)
                hp1_all = at.tile([128, NC * LT * 65], bf16)
                for r in range(NC):
                    nc.sync.dma_start(
                        out=hpT_all[:, r * LP:(r + 1) * LP],
                        in_=ag1_out[r:r + 1, 0:HPT_SZ].rearrange(
                            "o (p n) -> (o p) n", p=F))
                    nc.sync.dma_start(
                        out=hp1_all[:, r * LT * 65:(r + 1) * LT * 65],
                        in_=ag1_out[r:r + 1, HPT_SZ:AG1N].rearrange(
                            "o (p n) -> (o p) n", p=128))
                for b in range(B):
                    expT_all = at.tile([128, LT * YSP], bf16, tag="expT", bufs=2)
                    for lt in range(LT):
                        psS = atps.tile([128, YSP], f32, tag="S")
                        for c0, cw in ((0, 512), (512, 512), (1024, 128)):
                            nc.tensor.matmul(
                                psS[:, c0:c0 + cw],
                                lhsT=hpT_all[:, b * LP + lt * 128: b * LP + (lt + 1) * 128],
                                rhs=u4t_sb[:, c0:c0 + cw],
                                start=True, stop=True)
                        nc.scalar.activation(
                            out=expT_all[:, lt * YSP:(lt + 1) * YSP],
                            in_=psS[:], func=Exp,
                            bias=(expmask_sb[:, 0:1] if lt == LT - 1 else 0.0))
                    for c0, cw in ((0, 512), (512, 512), (1024, 128)):
                        psM = atps1.tile([65, 512], f32, tag="M")
                        for lt in range(LT):
                            nc.tensor.matmul(
                                psM[:, 0:cw],
                                lhsT=hp1_all[:, (b * LT + lt) * 65:(b * LT + lt + 1) * 65],
                                rhs=expT_all[:, lt * YSP + c0: lt * YSP + c0 + cw],
                                start=(lt == 0), stop=(lt == LT - 1))
                        inv_sb = at.tile([1, 512], bf16, tag="inv", bufs=2)
                        with nc.allow_low_precision(reason="softmax denom bf16"):
                            nc.vector.reciprocal(out=inv_sb[:, 0:cw],
                                                 in_=psM[64:65, 0:cw])
                        psB = atps1.tile([F, 512], f32, tag="Bc")
                        nc.tensor.matmul(psB[:, 0:cw], lhsT=ones_sb[:],
                                         rhs=inv_sb[:, 0:cw], start=True, stop=True)
                        bcast_sb = at.tile([F, 512], bf16, tag="bcast", bufs=2)
                        nc.scalar.copy(out=bcast_sb[:, 0:cw], in_=psB[:, 0:cw])
                        nc.vector.tensor_tensor(
                            out=m4tT_sb[:, b * YSP + c0: b * YSP + c0 + cw],
                            in0=psM[0:F, 0:cw], in1=bcast_sb[:, 0:cw],
                            op=Mult)
                        with nc.allow_low_precision(reason="gcn path fp8"):
                            nc.vector.tensor_tensor(
                                out=m4tf8_sb[:, b * YSP + c0: b * YSP + c0 + cw],
                                in0=psM[0:F, 0:cw], in1=bcast_sb[:, 0:cw],
                                op=Mult)
            if DBG:
                nc.sync.dma_start(out=dbg_m4t[:], in_=m4tT_sb[:])
            nc.sync.dma_start(
                out=ag2_in[:].rearrange("(p n) -> p n", p=F), in_=m4tf8_sb[:])
            nc.gpsimd.collective_compute(
                "AllGather", mybir.AluOpType.bypass, replica_groups=RG,
                ins=[ag2_in.opt()], outs=[ag2_out.opt()])

            # ---------------- phase 3: graph conv + label scoring ----------------
            with tc.tile_pool(name="p3", bufs=1) as p3, \
                 tc.tile_pool(name="p3m", bufs=2) as p3m, \
                 tc.tile_pool(name="p3ps", bufs=2, space="PSUM") as p3ps:
                # final-layer weights repeated per batch slot (done once, on-chip)
                f4tw8 = p3.tile([128, YT * NBG], bf16)
                f4w18 = p3.tile([128, YT * NBG], bf16)
                f4w28 = p3.tile([128, YT * NBG], bf16)
                for dst, src in ((f4tw8, f4tw_sb), (f4w18, f4w1_sb), (f4w28, f4w2_sb)):
                    dv = dst[:].rearrange("p (t b f) -> p t b f", t=YT, b=B)
                    sv = src[:].rearrange("p (t f) -> p t f", t=YT)
                    for b in range(B):
                        nc.scalar.copy(out=dv[:, :, b, :], in_=sv)

                if DBG:
                    nc.sync.dma_start(out=dbg_f4[:], in_=f4tw8[:])
                supp_sb = p3.tile([128, ZT * NBG], fp8)
                # m4t full, paired batches stacked on partitions for block-diag gcn
                ag2v = ag2_out.rearrange("r (f b n) -> f r b n", f=F, b=B)
                with tc.tile_pool(name="p3mp", bufs=1) as p3mp:
                    mpall = p3mp.tile([2 * F, 4 * ZPAD], fp8)
                    for pair in range(B // 2):
                        b0 = 2 * pair
                        nc.sync.dma_start(
                            out=mpall[0:F, pair * ZPAD:(pair + 1) * ZPAD].rearrange(
                                "p (r o n) -> p r o n", r=NC, o=1),
                            in_=ag2v[:, :, b0:b0 + 1, :])
                        nc.sync.dma_start(
                            out=mpall[F:2 * F, pair * ZPAD:(pair + 1) * ZPAD].rearrange(
                                "p (r o n) -> p r o n", r=NC, o=1),
                            in_=ag2v[:, :, b0 + 1:b0 + 2, :])
                    for zt in range(ZT):
                        psU = p3ps.tile([128, NBG], f32, tag="U")
                        for pair in range(B // 2):
                            nc.tensor.matmul(
                                psU[:, pair * 100:(pair + 1) * 100],
                                lhsT=mpall[:, pair * ZPAD + zt * 128:
                                           pair * ZPAD + (zt + 1) * 128],
                                rhs=gcn2_sb[:], start=True, stop=True)
                        with nc.allow_low_precision(reason="gcn support fp8"):
                            nc.vector.tensor_copy(
                                out=supp_sb[:, zt * NBG:(zt + 1) * NBG], in_=psU[:])

                if DBG:
                    nc.sync.dma_start(out=dbg_supp[:], in_=supp_sb[:])
                suppv = supp_sb[:].rearrange("p (z n) -> p z n", n=NBG)
                for yt in range(YT):
                    psO = p3ps.tile([128, NBG], f32, tag="O")
                    for zh in range(2):
                        stripe8 = p3m.tile([128, 36 * 128], fp8, tag="adj8")
                        nc.sync.dma_start(
                            out=stripe8[:],
                            in_=adjt[yt:yt + 1, zh:zh + 1].rearrange(
                                "o q p n -> (o q p) n"))
                        sv = stripe8[:].rearrange("p (t y) -> p t y", y=128)
                        if DR:
                            for tl2 in range(18):
                                zt0 = zh * 36 + 2 * tl2
                                nc.tensor.matmul(
                                    psO[:],
                                    lhsT=sv[:, 2 * tl2:2 * tl2 + 2, :],
                                    rhs=suppv[:, zt0:zt0 + 2, :],
                                    start=(zt0 == 0), stop=(zt0 + 2 == ZT),
                                    perf_mode=mybir.MatmulPerfMode.DoubleRow)
                        else:
                            for tl in range(36):
                                zt = zh * 36 + tl
                                nc.tensor.matmul(
                                    psO[:],
                                    lhsT=stripe8[:, tl * 128:(tl + 1) * 128],
                                    rhs=supp_sb[:, zt * NBG:(zt + 1) * NBG],
                                    start=(zt == 0), stop=(zt == ZT - 1))
                    o1 = p3.tile([128, NBG], f32, tag="o1", bufs=2)
                    o2 = p3.tile([128, NBG], f32, tag="o2", bufs=2)
                    nc.vector.tensor_tensor(out=o1[:], in0=psO[:], in1=gcnb_sb[:],
                                            op=Add)
                    nc.vector.tensor_scalar_mul(o2[:], o1[:], 0.2)
                    nc.vector.tensor_tensor(out=o1[:], in0=o1[:], in1=o2[:], op=Max)
                    # batched per-label scoring over all 8 batches at once
                    m4t_all = p3.tile([128, NBG], bf16, tag="m4a", bufs=2)
                    for b in range(B):
                        psT = p3ps.tile([128, 64], bf16, tag="T")
                        nc.tensor.transpose(
                            psT[:, 0:F],
                            m4tT_sb[:, b * YSP + yt * 128: b * YSP + (yt + 1) * 128],
                            identbf_sb[0:F, 0:F])
                        nc.scalar.copy(out=m4t_all[:, b * F:(b + 1) * F],
                                       in_=psT[:, 0:F])
                    scr = p3.tile([128, NBG], f32, tag="scr", bufs=2)
                    acc1 = p3.tile([128, B], f32, tag="a1", bufs=2)
                    acc2 = p3.tile([128, B], f32, tag="a2", bufs=2)
                    stage = p3.tile([128, 16], f32, tag="stage", bufs=2)
                    nc.vector.tensor_tensor(
                        out=scr[:], in0=m4t_all[:],
                        in1=f4tw8[:, yt * NBG:(yt + 1) * NBG], op=Mult)
                    nc.vector.reduce_sum(
                        out=acc1[:], in_=scr[:].rearrange("p (b f) -> p b f", b=B),
                        axis=mybir.AxisListType.X)
                    nc.vector.tensor_scalar(
                        out=stage[:, 0:8], in0=acc1[:],
                        scalar1=b4t_sb[:, yt:yt + 1], scalar2=None, op0=Add)
                    nc.vector.tensor_tensor(
                        out=scr[:], in0=m4t_all[:],
                        in1=f4w18[:, yt * NBG:(yt + 1) * NBG], op=Mult)
                    nc.vector.reduce_sum(
                        out=acc1[:], in_=scr[:].rearrange("p (b f) -> p b f", b=B),
                        axis=mybir.AxisListType.X)
                    nc.vector.tensor_tensor(
                        out=scr[:], in0=o1[:],
                        in1=f4w28[:, yt * NBG:(yt + 1) * NBG], op=Mult)
                    nc.vector.reduce_sum(
                        out=acc2[:], in_=scr[:].rearrange("p (b f) -> p b f", b=B),
                        axis=mybir.AxisListType.X)
                    nc.vector.tensor_tensor(out=acc1[:], in0=acc1[:], in1=acc2[:],
                                            op=Add)
                    nc.vector.tensor_scalar(
                        out=stage[:, 8:16], in0=acc1[:],
                        scalar1=b4_sb[:, yt:yt + 1], scalar2=None, op0=Add)
                    if DBG and yt == 0:
                        nc.sync.dma_start(out=dbg_m4a[:], in_=m4t_all[:])
                    nc.sync.dma_start(out=outc[yt * 128:(yt + 1) * 128, :],
                                      in_=stage[:])

    nc.compile()
    return nc


def _bf(x):
    return np.ascontiguousarray(np.asarray(x, dtype=np.float32).astype(BF16))


def _prep_inputs(x, embed_w, conv_w, conv_b, U4_w, gcn_w, gcn_b, adj,
                 final4t_w, final4t_b, final4_w, final4_b):
    x = np.asarray(x).astype(np.int64)
    embed_w = np.asarray(embed_w, dtype=np.float32)
    conv_w = np.asarray(conv_w, dtype=np.float32)
    conv_b = np.asarray(conv_b, dtype=np.float32)
    U4_w = np.asarray(U4_w, dtype=np.float32)
    gcn_w = np.asarray(gcn_w, dtype=np.float32)
    gcn_b = np.asarray(gcn_b, dtype=np.float32)
    adj = np.asarray(adj, dtype=np.float32)
    f4t_w = np.asarray(final4t_w, dtype=np.float32)
    f4t_b = np.asarray(final4t_b, dtype=np.float32)
    f4_w = np.asarray(final4_w, dtype=np.float32)
    f4_b = np.asarray(final4_b, dtype=np.float32)

    conv_lhsT = np.zeros((E, KS * F), np.float32)
    for k in range(KS):
        conv_lhsT[:, k * F:(k + 1) * F] = conv_w[:, :, k].T
    conv_lhsT = _bf(conv_lhsT)
    conv_bias = np.ascontiguousarray(conv_b.reshape(F, 1))

    # adj scale: power of two s with s*max(adj) <= 224 (TRN fp8e4 max 240);
    # 1/s is folded into the gcn weight so the device GEMM is exact.
    amax = float(np.abs(adj).max()) or 1.0
    s = 2.0 ** np.floor(np.log2(224.0 / amax))
    gcn2 = np.zeros((2 * F, 2 * F), np.float32)
    gcn2[:F, :F] = gcn_w
    gcn2[F:, F:] = gcn_w
    gcn2 = _bf(gcn2)
    # leaky_relu is positive-homogeneous: leaky(s*x + s*b) = s*leaky(x+b), so
    # scale the bias by s here and fold 1/s into the final4 out1-half weights.
    gcnb_bc = np.ascontiguousarray(
        np.broadcast_to(np.tile(gcn_b * s, B)[None, :], (128, NBG)))
    identbf = _bf(np.eye(128, dtype=np.float32))
    expmask = np.zeros((128, 1), np.float32)
    expmask[L - (LT - 1) * 128:, 0] = -30000.0
    ones50 = _bf(np.ones((1, F), np.float32))

    shared = dict(conv_lhsT=conv_lhsT, conv_bias=conv_bias,
                  gcn2=gcn2, gcnb_bc=gcnb_bc, identbf=identbf,
                  expmask=expmask, ones50=ones50)

    in_maps = []
    for c in range(NC):
        v = VALID[c]
        embT = np.zeros((128, LP + 8), np.float32)
        embT[:E, 4:4 + L] = embed_w[x[c]].T
        embT = _bf(embT)

        u4t_c = np.zeros((F, YSP), np.float32)
        u4t_c[:, :v] = U4_w[c * YSV:c * YSV + v].T

        at = np.zeros((ZPAD, YSP), np.float32)
        for blk in range(NC):
            vb = VALID[blk]
            at[blk * YSP:blk * YSP + vb, :v] = adj[c * YSV:c * YSV + v,
                                                   blk * YSV:blk * YSV + vb].T
        at = np.clip(at * s, 0.0, 240.0).astype(FP8)
        # stripe-contiguous: [yt, zh, zp, t, y] so each [128, 36*128] stripe
        # DMA is one 4608B descriptor per partition
        adjt_c = np.ascontiguousarray(
            at.reshape(2, 36, 128, YT, 128).transpose(3, 0, 2, 1, 4)
            .reshape(YT, 2, 128, 36 * 128))

        def rowpack(w):
            out = np.zeros((128, YT * F), np.float32)
            wp = np.zeros((YSP, F), np.float32)
            wp[:v] = w[c * YSV:c * YSV + v]
            for yt in range(YT):
                out[:, yt * F:(yt + 1) * F] = wp[yt * 128:(yt + 1) * 128]
            return _bf(out)

        def biaspack(bias):
            out = np.zeros((128, YT), np.float32)
            bp = np.zeros(YSP, np.float32)
            bp[:v] = bias[c * YSV:c * YSV + v]
            out[:, :] = bp.reshape(YT, 128).T
            return np.ascontiguousarray(out)

        m = dict(shared)
        m.update(embT=embT, u4t=_bf(u4t_c), adjt=adjt_c,
                 f4tw=rowpack(f4t_w), f4w1=rowpack(f4_w[:, :F]),
                 f4w2=rowpack(f4_w[:, F:] / s), b4t=biaspack(f4t_b),
                 b4=biaspack(f4_b))
        in_maps.append(m)
    return in_maps


def _postprocess(results):
    y4t = np.zeros((B, Y), np.float32)
    y4 = np.zeros((B, Y), np.float32)
    for c in range(NC):
        v = VALID[c]
        oc = results[c]["outc"]
        y4t[:, c * YSV:c * YSV + v] = oc[:v, 0:8].T
        y4[:, c * YSV:c * YSV + v] = oc[:v, 8:16].T
    return y4t, y4


def _get_nc():
    if "nc" not in _CACHE:
        _CACHE["nc"] = _build()
    return _CACHE["nc"]


def run_raw(in_maps, **kw):
    nc = _get_nc()
    return run_bass_kernel_spmd(nc, in_maps, list(range(NC)), **kw)


def kernel(x, target, embed_w, conv_w, conv_b, U4_w, gcn_w, gcn_b, adj,
           final4t_w, final4t_b, final4_w, final4_b):
    in_maps = _prep_inputs(x, embed_w, conv_w, conv_b, U4_w, gcn_w, gcn_b, adj,
                           final4t_w, final4t_b, final4_w, final4_b)
    res = run_raw(in_maps)
    return _postprocess(res.results)


# revision 24
# speedup vs baseline: 10171.1855x; 1.0143x over previous
"""Trainium2 Bass kernel for nn_ConvAttnPool (conv + per-label attention pooling
+ label-graph conv + label-wise scoring), SPMD over 8 NeuronCores.

Sharding: label dim Y=8922 is split 8 ways (1116/core, padded to 1152); the
front-end conv is sharded over batch (1 batch/core) followed by an AllGather of
the conv activations; a second AllGather exchanges the per-label pooled
features (m4t) for the graph conv.

Host-side prep keeps the per-call device payload small (the axon PJRT tunnel
is ~70 MB/s): the embedding lookup is resolved on host (ship the 0.66 MB
gathered+transposed activations instead of the 25.6 MB vocab table per core),
and the adjacency is pre-transposed, stripe-contiguous (one DMA descriptor per
partition), scaled by a power of two s and stored fp8e4 (TRN E4M3, max 240) —
the matching 1/s is folded into the gcn weight so the device computes exactly
adj @ support with no extra ops.
"""
import numpy as np
import ml_dtypes

import concourse.bass as bass
import concourse.bacc as bacc
import concourse.tile as tile
from concourse import mybir
from concourse.bass_utils import run_bass_kernel_spmd

BF16 = ml_dtypes.bfloat16
FP8 = ml_dtypes.float8_e4m3

# problem dims (hardcoded per contract)
B, L, V, E, F, KS, Y = 8, 2500, 50002, 100, 50, 9, 8922
NC = 8
YSV = 1116                   # labels per core (last core has 1110 valid)
YSP = 1152                   # padded labels per core
YT = YSP // 128              # 9 y-tiles
LP = 2560                    # padded seq len
LT = LP // 128               # 20 l-tiles
ZPAD = NC * YSP              # 9216 padded global label dim
ZT = ZPAD // 128             # 72 z-tiles
NBG = B * F                  # 400
VALID = [YSV] * (NC - 1) + [Y - (NC - 1) * YSV]

f32 = mybir.dt.float32
bf16 = mybir.dt.bfloat16
fp8 = mybir.dt.float8e4
i32 = mybir.dt.int32

_CACHE = {}
import os
ADJ_MIXED = int(os.environ.get('K_ADJ_MIXED', '1'))
DR = int(os.environ.get('K_DR', '1'))
DBG = int(os.environ.get('K_DBG', '0'))


def _build():
    nc = bacc.Bacc("TRN2", target_bir_lowering=False, debug=False,
                   enable_asserts=True, num_devices=NC)

    embT = nc.dram_tensor("embT", [128, LP + 8], bf16, kind="ExternalInput")
    conv_lhsT = nc.dram_tensor("conv_lhsT", [E, KS * F], bf16, kind="ExternalInput")
    conv_bias = nc.dram_tensor("conv_bias", [F, 1], f32, kind="ExternalInput")
    u4t = nc.dram_tensor("u4t", [F, YSP], bf16, kind="ExternalInput")
    adjt = nc.dram_tensor("adjt", [YT, 2, 128, 36 * 128], fp8, kind="ExternalInput")
    gcn2 = nc.dram_tensor("gcn2", [2 * F, 2 * F], bf16, kind="ExternalInput")
    gcnb_bc = nc.dram_tensor("gcnb_bc", [128, NBG], f32, kind="ExternalInput")
    f4tw = nc.dram_tensor("f4tw", [128, YT * F], bf16, kind="ExternalInput")
    f4w1 = nc.dram_tensor("f4w1", [128, YT * F], bf16, kind="ExternalInput")
    f4w2 = nc.dram_tensor("f4w2", [128, YT * F], bf16, kind="ExternalInput")
    b4t = nc.dram_tensor("b4t", [128, YT], f32, kind="ExternalInput")
    b4 = nc.dram_tensor("b4", [128, YT], f32, kind="ExternalInput")
    identbf = nc.dram_tensor("identbf", [128, 128], bf16, kind="ExternalInput")
    expmask = nc.dram_tensor("expmask", [128, 1], f32, kind="ExternalInput")
    ones50 = nc.dram_tensor("ones50", [1, F], bf16, kind="ExternalInput")
    outc = nc.dram_tensor("outc", [YSP, 16], f32, kind="ExternalOutput")
    if DBG:
        dbg_m4t = nc.dram_tensor("dbg_m4t", [F, B * YSP], bf16, kind="ExternalOutput")
        dbg_m4a = nc.dram_tensor("dbg_m4a", [128, NBG], bf16, kind="ExternalOutput")
        dbg_f4 = nc.dram_tensor("dbg_f4", [128, YT * NBG], bf16, kind="ExternalOutput")
        dbg_supp = nc.dram_tensor("dbg_supp", [128, ZT * NBG], fp8, kind="ExternalOutput")
        dbg_hp1 = nc.dram_tensor("dbg_hp1", [128, LT * 65], bf16, kind="ExternalOutput")
        dbg_embT = nc.dram_tensor("dbg_embT", [128, LP + 8], bf16, kind="ExternalOutput")
        dbg_hpT = nc.dram_tensor("dbg_hpT", [F, LP], bf16, kind="ExternalOutput")

    Exp = mybir.ActivationFunctionType.Exp
    Tanh = mybir.ActivationFunctionType.Tanh
    RG = [list(range(NC))]
    Mult = mybir.AluOpType.mult
    Add = mybir.AluOpType.add
    Max = mybir.AluOpType.max

    with tile.TileContext(nc) as tc:
        with tc.tile_pool(name="const", bufs=1) as cp, \
             tc.tile_pool(name="pers", bufs=1) as pers, \
             tc.tile_pool(name="dram", bufs=1, space="DRAM") as dram:
            # phase-1-critical consts first so the conv isn't queued behind
            # phase-2/3-only weight DMAs
            convw_sb = cp.tile([E, KS * F], bf16)
            nc.sync.dma_start(out=convw_sb[:], in_=conv_lhsT[:])
            convb_sb = cp.tile([F, 1], f32)
            nc.sync.dma_start(out=convb_sb[:], in_=conv_bias[:])
            identbf_sb = cp.tile([128, 128], bf16)
            nc.sync.dma_start(out=identbf_sb[:], in_=identbf[:])
            u4t_sb = cp.tile([F, YSP], bf16)
            nc.sync.dma_start(out=u4t_sb[:], in_=u4t[:])
            gcn2_sb = cp.tile([2 * F, 2 * F], bf16)
            nc.sync.dma_start(out=gcn2_sb[:], in_=gcn2[:])
            gcnb_sb = cp.tile([128, NBG], f32)
            nc.sync.dma_start(out=gcnb_sb[:], in_=gcnb_bc[:])
            f4tw_sb = cp.tile([128, YT * F], bf16)
            nc.sync.dma_start(out=f4tw_sb[:], in_=f4tw[:])
            f4w1_sb = cp.tile([128, YT * F], bf16)
            nc.sync.dma_start(out=f4w1_sb[:], in_=f4w1[:])
            f4w2_sb = cp.tile([128, YT * F], bf16)
            nc.sync.dma_start(out=f4w2_sb[:], in_=f4w2[:])
            b4t_sb = cp.tile([128, YT], f32)
            nc.sync.dma_start(out=b4t_sb[:], in_=b4t[:])
            b4_sb = cp.tile([128, YT], f32)
            nc.sync.dma_start(out=b4_sb[:], in_=b4[:])
            expmask_sb = cp.tile([128, 1], f32)
            nc.sync.dma_start(out=expmask_sb[:], in_=expmask[:])
            ones_sb = cp.tile([1, F], bf16)
            nc.sync.dma_start(out=ones_sb[:], in_=ones50[:])

            m4tT_sb = pers.tile([F, B * YSP], bf16)   # this core's label slice
            m4tf8_sb = pers.tile([F, B * YSP], fp8)   # fp8 twin for the gcn path

            HPT_SZ = F * LP               # 128000
            AG1N = HPT_SZ + 128 * LT * 65
            ag1_in = dram.tile([AG1N], bf16)
            ag1_out = dram.tile([NC, AG1N], bf16, addr_space="Shared")

            # ---------------- phase 1: conv on own batch ----------------
            with tc.tile_pool(name="p1", bufs=1) as p1, \
                 tc.tile_pool(name="p1ps", bufs=2, space="PSUM") as p1ps:
                embT_sb = p1.tile([128, LP + 8], bf16)
                nc.sync.dma_start(out=embT_sb[:], in_=embT[:])
                hpT_sb = p1.tile([F, LP], bf16)
                for l5 in range(5):
                    psc = p1ps.tile([F, 512], f32, tag="conv")
                    for k in range(KS):
                        nc.tensor.matmul(
                            psc[:],
                            lhsT=convw_sb[:, k * F:(k + 1) * F],
                            rhs=embT_sb[0:E, l5 * 512 + k: l5 * 512 + k + 512],
                            start=(k == 0), stop=(k == KS - 1))
                    nc.scalar.activation(out=hpT_sb[:, l5 * 512:(l5 + 1) * 512],
                                         in_=psc[:], func=Tanh,
                                         bias=convb_sb[:, 0:1])
                hp1_sb = p1.tile([128, LT * 65], bf16)
                nc.vector.memset(hp1_sb[:], 1.0)
                for g, gn in ((0, 8), (8, 8), (16, 4)):
                    pst2 = p1ps.tile([128, 8 * 64], bf16, tag="tp2")
                    for i in range(gn):
                        lt = g + i
                        nc.tensor.transpose(pst2[:, i * 64:i * 64 + F],
                                            hpT_sb[:, lt * 128:(lt + 1) * 128],
                                            identbf_sb[0:F, 0:F])
                    nc.scalar.copy(
                        out=hp1_sb[:].rearrange("p (l c) -> p l c", c=65)[:, g:g + gn, 0:F],
                        in_=pst2[:].rearrange("p (l c) -> p l c", c=64)[:, 0:gn, 0:F])
                if DBG:
                    nc.sync.dma_start(out=dbg_hp1[:], in_=hp1_sb[:])
                    nc.sync.dma_start(out=dbg_embT[:], in_=embT_sb[:])
                    nc.sync.dma_start(out=dbg_hpT[:], in_=hpT_sb[:])
                nc.sync.dma_start(
                    out=ag1_in[0:HPT_SZ].rearrange("(p n) -> p n", p=F),
                    in_=hpT_sb[:])
                nc.sync.dma_start(
                    out=ag1_in[HPT_SZ:AG1N].rearrange("(p n) -> p n", p=128),
                    in_=hp1_sb[:])
            nc.gpsimd.collective_compute(
                "AllGather", mybir.AluOpType.bypass, replica_groups=RG,
                ins=[ag1_in.opt()], outs=[ag1_out.opt()])

            ag2_in = dram.tile([F * B * YSP], fp8)
            ag2_out = dram.tile([NC, F * B * YSP], fp8, addr_space="Shared")

            # ---------------- phase 2: per-label attention ----------------
            with tc.tile_pool(name="attn", bufs=1) as at, \
                 tc.tile_pool(name="atps", bufs=2, space="PSUM") as atps, \
                 tc.tile_pool(name="atps1", bufs=1, space="PSUM") as atps1:
                hpT_all = at.tile([F, NC * LP], bf16)
                hp1_all = at.tile([128, NC * LT * 65], bf16)
                for r in range(NC):
                    nc.sync.dma_start(
                        out=hpT_all[:, r * LP:(r + 1) * LP],
                        in_=ag1_out[r:r + 1, 0:HPT_SZ].rearrange(
                            "o (p n) -> (o p) n", p=F))
                    nc.sync.dma_start(
                        out=hp1_all[:, r * LT * 65:(r + 1) * LT * 65],
                        in_=ag1_out[r:r + 1, HPT_SZ:AG1N].rearrange(
                            "o (p n) -> (o p) n", p=128))
                for b in range(B):
                    expT_all = at.tile([128, LT * YSP], bf16, tag="expT", bufs=2)
                    for lt in range(LT):
                        psS = atps.tile([128, YSP], f32, tag="S")
                        for c0, cw in ((0, 512), (512, 512), (1024, 128)):
                            nc.tensor.matmul(
                                psS[:, c0:c0 + cw],
                                lhsT=hpT_all[:, b * LP + lt * 128: b * LP + (lt + 1) * 128],
                                rhs=u4t_sb[:, c0:c0 + cw],
                                start=True, stop=True)
                        nc.scalar.activation(
                            out=expT_all[:, lt * YSP:(lt + 1) * YSP],
                            in_=psS[:], func=Exp,
                            bias=(expmask_sb[:, 0:1] if lt == LT - 1 else 0.0))
                    for c0, cw in ((0, 512), (512, 512), (1024, 128)):
                        psM = atps1.tile([65, 512], f32, tag="M")
                        for lt in range(LT):
                            nc.tensor.matmul(
                                psM[:, 0:cw],
                                lhsT=hp1_all[:, (b * LT + lt) * 65:(b * LT + lt + 1) * 65],
                                rhs=expT_all[:, lt * YSP + c0: lt * YSP + c0 + cw],
                                start=(lt == 0), stop=(lt == LT - 1))
                        inv_sb = at.tile([1, 512], bf16, tag="inv", bufs=2)
                        with nc.allow_low_precision(reason="softmax denom bf16"):
                            nc.vector.reciprocal(out=inv_sb[:, 0:cw],
                                                 in_=psM[64:65, 0:cw])
                        psB = atps1.tile([F, 512], f32, tag="Bc")
                        nc.tensor.matmul(psB[:, 0:cw], lhsT=ones_sb[:],
                                         rhs=inv_sb[:, 0:cw], start=True, stop=True)
                        bcast_sb = at.tile([F, 512], bf16, tag="bcast", bufs=2)
                        nc.scalar.copy(out=bcast_sb[:, 0:cw], in_=psB[:, 0:cw])
                        nc.vector.tensor_tensor(
                            out=m4tT_sb[:, b * YSP + c0: b * YSP + c0 + cw],
                            in0=psM[0:F, 0:cw], in1=bcast_sb[:, 0:cw],
                            op=Mult)
                        with nc.allow_low_precision(reason="gcn path fp8"):
                            nc.vector.tensor_tensor(
                                out=m4tf8_sb[:, b * YSP + c0: b * YSP + c0 + cw],
                                in0=psM[0:F, 0:cw], in1=bcast_sb[:, 0:cw],
                                op=Mult)
            if DBG:
                nc.sync.dma_start(out=dbg_m4t[:], in_=m4tT_sb[:])
            nc.sync.dma_start(
                out=ag2_in[:].rearrange("(p n) -> p n", p=F), in_=m4tf8_sb[:])
            nc.gpsimd.collective_compute(
                "AllGather", mybir.AluOpType.bypass, replica_groups=RG,
                ins=[ag2_in.opt()], outs=[ag2_out.opt()])

            # ---------------- phase 3: graph conv + label scoring ----------------
            with tc.tile_pool(name="p3", bufs=1) as p3, \
                 tc.tile_pool(name="p3m", bufs=2) as p3m, \
                 tc.tile_pool(name="p3ps", bufs=2, space="PSUM") as p3ps:
                # final-layer weights repeated per batch slot (done once, on-chip)
                f4tw8 = p3.tile([128, YT * NBG], bf16)
                f4w18 = p3.tile([128, YT * NBG], bf16)
                f4w28 = p3.tile([128, YT * NBG], bf16)
                for dst, src in ((f4tw8, f4tw_sb), (f4w18, f4w1_sb), (f4w28, f4w2_sb)):
                    dv = dst[:].rearrange("p (t b f) -> p t b f", t=YT, b=B)
                    sv = src[:].rearrange("p (t f) -> p t f", t=YT)
                    for b in range(B):
                        nc.scalar.copy(out=dv[:, :, b, :], in_=sv)

                if DBG:
                    nc.sync.dma_start(out=dbg_f4[:], in_=f4tw8[:])
                supp_sb = p3.tile([128, ZT * NBG], fp8)
                # m4t full, paired batches stacked on partitions for block-diag gcn
                ag2v = ag2_out.rearrange("r (f b n) -> f r b n", f=F, b=B)
                with tc.tile_pool(name="p3mp", bufs=1) as p3mp:
                    mpall = p3mp.tile([2 * F, 4 * ZPAD], fp8)
                    for pair in range(B // 2):
                        b0 = 2 * pair
                        nc.sync.dma_start(
                            out=mpall[0:F, pair * ZPAD:(pair + 1) * ZPAD].rearrange(
                                "p (r o n) -> p r o n", r=NC, o=1),
                            in_=ag2v[:, :, b0:b0 + 1, :])
                        nc.sync.dma_start(
                            out=mpall[F:2 * F, pair * ZPAD:(pair + 1) * ZPAD].rearrange(
                                "p (r o n) -> p r o n", r=NC, o=1),
                            in_=ag2v[:, :, b0 + 1:b0 + 2, :])
                    for zt in range(ZT):
                        psU = p3ps.tile([128, NBG], f32, tag="U")
                        for pair in range(B // 2):
                            nc.tensor.matmul(
                                psU[:, pair * 100:(pair + 1) * 100],
                                lhsT=mpall[:, pair * ZPAD + zt * 128:
                                           pair * ZPAD + (zt + 1) * 128],
                                rhs=gcn2_sb[:], start=True, stop=True)
                        with nc.allow_low_precision(reason="gcn support fp8"):
                            nc.vector.tensor_copy(
                                out=supp_sb[:, zt * NBG:(zt + 1) * NBG], in_=psU[:])

                if DBG:
                    nc.sync.dma_start(out=dbg_supp[:], in_=supp_sb[:])
                suppv = supp_sb[:].rearrange("p (z n) -> p z n", n=NBG)
                for yt in range(YT):
                    psO = p3ps.tile([128, NBG], f32, tag="O")
                    for zh in range(2):
                        stripe8 = p3m.tile([128, 36 * 128], fp8, tag="adj8")
                        nc.sync.dma_start(
                            out=stripe8[:],
                            in_=adjt[yt:yt + 1, zh:zh + 1].rearrange(
                                "o q p n -> (o q p) n"))
                        sv = stripe8[:].rearrange("p (t y) -> p t y", y=128)
                        if DR:
                            for tl2 in range(18):
                                zt0 = zh * 36 + 2 * tl2
                                nc.tensor.matmul(
                                    psO[:],
                                    lhsT=sv[:, 2 * tl2:2 * tl2 + 2, :],
                                    rhs=suppv[:, zt0:zt0 + 2, :],
                                    start=(zt0 == 0), stop=(zt0 + 2 == ZT),
                                    perf_mode=mybir.MatmulPerfMode.DoubleRow)
                        else:
                            for tl in range(36):
                                zt = zh * 36 + tl
                                nc.tensor.matmul(
                                    psO[:],
                                    lhsT=stripe8[:, tl * 128:(tl + 1) * 128],
                                    rhs=supp_sb[:, zt * NBG:(zt + 1) * NBG],
                                    start=(zt == 0), stop=(zt == ZT - 1))
                    o1 = p3.tile([128, NBG], f32, tag="o1", bufs=2)
                    o2 = p3.tile([128, NBG], f32, tag="o2", bufs=2)
                    nc.vector.tensor_tensor(out=o1[:], in0=psO[:], in1=gcnb_sb[:],
                                            op=Add)
                    nc.vector.tensor_scalar_mul(o2[:], o1[:], 0.2)
                    nc.vector.tensor_tensor(out=o1[:], in0=o1[:], in1=o2[:], op=Max)
                    # batched per-label scoring over all 8 batches at once
                    m4t_all = p3.tile([128, NBG], bf16, tag="m4a", bufs=2)
                    psT = p3ps.tile([128, B * 64], bf16, tag="T")
                    for b in range(B):
                        nc.tensor.transpose(
                            psT[:, b * 64:b * 64 + F],
                            m4tT_sb[:, b * YSP + yt * 128: b * YSP + (yt + 1) * 128],
                            identbf_sb[0:F, 0:F])
                    nc.scalar.copy(
                        out=m4t_all[:].rearrange("p (b f) -> p b f", b=B),
                        in_=psT[:].rearrange("p (b g) -> p b g", b=B)[:, :, 0:F])
                    scr = p3.tile([128, NBG], f32, tag="scr", bufs=2)
                    acc1 = p3.tile([128, B], f32, tag="a1", bufs=2)
                    acc2 = p3.tile([128, B], f32, tag="a2", bufs=2)
                    stage = p3.tile([128, 16], f32, tag="stage", bufs=2)
                    nc.vector.tensor_tensor(
                        out=scr[:], in0=m4t_all[:],
                        in1=f4tw8[:, yt * NBG:(yt + 1) * NBG], op=Mult)
                    nc.vector.reduce_sum(
                        out=acc1[:], in_=scr[:].rearrange("p (b f) -> p b f", b=B),
                        axis=mybir.AxisListType.X)
                    nc.vector.tensor_scalar(
                        out=stage[:, 0:8], in0=acc1[:],
                        scalar1=b4t_sb[:, yt:yt + 1], scalar2=None, op0=Add)
                    nc.vector.tensor_tensor(
                        out=scr[:], in0=m4t_all[:],
                        in1=f4w18[:, yt * NBG:(yt + 1) * NBG], op=Mult)
                    nc.vector.reduce_sum(
                        out=acc1[:], in_=scr[:].rearrange("p (b f) -> p b f", b=B),
                        axis=mybir.AxisListType.X)
                    nc.vector.tensor_tensor(
                        out=scr[:], in0=o1[:],
                        in1=f4w28[:, yt * NBG:(yt + 1) * NBG], op=Mult)
                    nc.vector.reduce_sum(
                        out=acc2[:], in_=scr[:].rearrange("p (b f) -> p b f", b=B),
                        axis=mybir.AxisListType.X)
                    nc.vector.tensor_tensor(out=acc1[:], in0=acc1[:], in1=acc2[:],
                                            op=Add)
                    nc.vector.tensor_scalar(
                        out=stage[:, 8:16], in0=acc1[:],
                        scalar1=b4_sb[:, yt:yt + 1], scalar2=None, op0=Add)
                    if DBG and yt == 0:
                        nc.sync.dma_start(out=dbg_m4a[:], in_=m4t_all[:])
                    nc.sync.dma_start(out=outc[yt * 128:(yt + 1) * 128, :],
                                      in_=stage[:])

    nc.compile()
    return nc


def _bf(x):
    return np.ascontiguousarray(np.asarray(x, dtype=np.float32).astype(BF16))


def _prep_inputs(x, embed_w, conv_w, conv_b, U4_w, gcn_w, gcn_b, adj,
                 final4t_w, final4t_b, final4_w, final4_b):
    x = np.asarray(x).astype(np.int64)
    embed_w = np.asarray(embed_w, dtype=np.float32)
    conv_w = np.asarray(conv_w, dtype=np.float32)
    conv_b = np.asarray(conv_b, dtype=np.float32)
    U4_w = np.asarray(U4_w, dtype=np.float32)
    gcn_w = np.asarray(gcn_w, dtype=np.float32)
    gcn_b = np.asarray(gcn_b, dtype=np.float32)
    adj = np.asarray(adj, dtype=np.float32)
    f4t_w = np.asarray(final4t_w, dtype=np.float32)
    f4t_b = np.asarray(final4t_b, dtype=np.float32)
    f4_w = np.asarray(final4_w, dtype=np.float32)
    f4_b = np.asarray(final4_b, dtype=np.float32)

    conv_lhsT = np.zeros((E, KS * F), np.float32)
    for k in range(KS):
        conv_lhsT[:, k * F:(k + 1) * F] = conv_w[:, :, k].T
    conv_lhsT = _bf(conv_lhsT)
    conv_bias = np.ascontiguousarray(conv_b.reshape(F, 1))

    # adj scale: power of two s with s*max(adj) <= 224 (TRN fp8e4 max 240);
    # 1/s is folded into the gcn weight so the device GEMM is exact.
    amax = float(np.abs(adj).max()) or 1.0
    s = 2.0 ** np.floor(np.log2(224.0 / amax))
    gcn2 = np.zeros((2 * F, 2 * F), np.float32)
    gcn2[:F, :F] = gcn_w
    gcn2[F:, F:] = gcn_w
    gcn2 = _bf(gcn2)
    # leaky_relu is positive-homogeneous: leaky(s*x + s*b) = s*leaky(x+b), so
    # scale the bias by s here and fold 1/s into the final4 out1-half weights.
    gcnb_bc = np.ascontiguousarray(
        np.broadcast_to(np.tile(gcn_b * s, B)[None, :], (128, NBG)))
    identbf = _bf(np.eye(128, dtype=np.float32))
    expmask = np.zeros((128, 1), np.float32)
    expmask[L - (LT - 1) * 128:, 0] = -30000.0
    ones50 = _bf(np.ones((1, F), np.float32))

    shared = dict(conv_lhsT=conv_lhsT, conv_bias=conv_bias,
                  gcn2=gcn2, gcnb_bc=gcnb_bc, identbf=identbf,
                  expmask=expmask, ones50=ones50)

    in_maps = []
    for c in range(NC):
        v = VALID[c]
        embT = np.zeros((128, LP + 8), np.float32)
        embT[:E, 4:4 + L] = embed_w[x[c]].T
        embT = _bf(embT)

        u4t_c = np.zeros((F, YSP), np.float32)
        u4t_c[:, :v] = U4_w[c * YSV:c * YSV + v].T

        at = np.zeros((ZPAD, YSP), np.float32)
        for blk in range(NC):
            vb = VALID[blk]
            at[blk * YSP:blk * YSP + vb, :v] = adj[c * YSV:c * YSV + v,
                                                   blk * YSV:blk * YSV + vb].T
        at = np.clip(at * s, 0.0, 240.0).astype(FP8)
        # stripe-contiguous: [yt, zh, zp, t, y] so each [128, 36*128] stripe
        # DMA is one 4608B descriptor per partition
        adjt_c = np.ascontiguousarray(
            at.reshape(2, 36, 128, YT, 128).transpose(3, 0, 2, 1, 4)
            .reshape(YT, 2, 128, 36 * 128))

        def rowpack(w):
            out = np.zeros((128, YT * F), np.float32)
            wp = np.zeros((YSP, F), np.float32)
            wp[:v] = w[c * YSV:c * YSV + v]
            for yt in range(YT):
                out[:, yt * F:(yt + 1) * F] = wp[yt * 128:(yt + 1) * 128]
            return _bf(out)

        def biaspack(bias):
            out = np.zeros((128, YT), np.float32)
            bp = np.zeros(YSP, np.float32)
            bp[:v] = bias[c * YSV:c * YSV + v]
            out[:, :] = bp.reshape(YT, 128).T
            return np.ascontiguousarray(out)

        m = dict(shared)
        m.update(embT=embT, u4t=_bf(u4t_c), adjt=adjt_c,
                 f4tw=rowpack(f4t_w), f4w1=rowpack(f4_w[:, :F]),
                 f4w2=rowpack(f4_w[:, F:] / s), b4t=biaspack(f4t_b),
                 b4=biaspack(f4_b))
        in_maps.append(m)
    return in_maps


def _postprocess(results):
    y4t = np.zeros((B, Y), np.float32)
    y4 = np.zeros((B, Y), np.float32)
    for c in range(NC):
        v = VALID[c]
        oc = results[c]["outc"]
        y4t[:, c * YSV:c * YSV + v] = oc[:v, 0:8].T
        y4[:, c * YSV:c * YSV + v] = oc[:v, 8:16].T
    return y4t, y4


def _get_nc():
    if "nc" not in _CACHE:
        _CACHE["nc"] = _build()
    return _CACHE["nc"]


def run_raw(in_maps, **kw):
    nc = _get_nc()
    return run_bass_kernel_spmd(nc, in_maps, list(range(NC)), **kw)


def kernel(x, target, embed_w, conv_w, conv_b, U4_w, gcn_w, gcn_b, adj,
           final4t_w, final4t_b, final4_w, final4_b):
    in_maps = _prep_inputs(x, embed_w, conv_w, conv_b, U4_w, gcn_w, gcn_b, adj,
                           final4t_w, final4t_b, final4_w, final4_b)
    res = run_raw(in_maps)
    return _postprocess(res.results)
